# revision 7
# baseline (speedup 1.0000x reference)
"""CQT (constant-Q transform) kernel for Trainium2, 8 NeuronCores.

Math: out[b, c, t] = sum_l W[c, l] * x_pad[b, t*HOP + l]   (strided conv,
HOP=512, L=11339 taps, C=168 channels = 84 bins x re/im), then reshaped to
(B, 2, n_bins, T_out).

Strategy (two-level factorization, data-parallel over batch):
  - Write l = 512q + s.  Level 1 contracts s: with the polyphase matrix
    Y[s, u] = xp[512u + s], compute G[(c,q), u] = sum_s W[c,512q+s] Y[s,u]
    for the ~815 ACTIVE (channel, hop-block) pairs only (CQT kernels are
    ragged: bin k has ~11339*2^(-k/12) centered taps, so sum_c ceil(l_c/512)
    ~= 815 of 168*23 possible pairs).  Packed into ceil(815/128) = 7 dense
    chunks of 128 pairs -> 28 matmuls per u-sweep at N=512, ~91% PE
    utilization (vs 29% for the naive 128-tap-block decomposition).
  - Shift on evict: PSUM->SBUF copies write Gs[p, t] = G[p, t+q(p)] (pairs
    sharing q form contiguous runs, so each run is one affine copy),
    converting fp32->fp16.
  - Level 2 contracts q: out[c, t] = sum_p E[p, c] Gs[p, t] with one-hot
    E per chunk -- 7+1 matmuls of N=512 per t-tile into the usual
    [C, t] PSUM layout.
  - fp16 x and 2^wexp-scaled fp16 W (undone on host); fp32 PSUM.
  - A formulation (weights stationary, x moving, fp32r) kept as fallback
    if the kernel tensor has no zero raggedness.
"""

import numpy as np

HOP = 512
N_CORES = 8

_prog_cache: dict = {}


def _host_prep(x, kernels):
    x = np.ascontiguousarray(np.asarray(x, dtype=np.float32))
    kernels = np.ascontiguousarray(np.asarray(kernels, dtype=np.float32))
    B, T = x.shape
    nbins, two, Lmax = kernels.shape
    assert two == 2
    C = 2 * nbins
    pad = Lmax // 2
    T_out = (T + 2 * pad - Lmax) // HOP + 1

    nblk_full = -(-Lmax // 128)
    nq = -(-(nblk_full * 128) // 512)
    Wp = np.zeros((C, nq * 512), dtype=np.float32)
    Wp[:, :Lmax] = kernels.reshape(C, Lmax)

    # ---- active (q, c) pairs for the D formulation ----
    seg_nz = (Wp.reshape(C, nq, 512) != 0.0).any(axis=2)  # [C, nq]
    # cluster c>=128 pairs at the end so they occupy the fewest chunks
    # (each chunk holding such pairs costs an extra matmul per t-tile)
    pairs = sorted(
        ((q, c) for q in range(nq) for c in range(C) if seg_nz[c, q]),
        key=lambda qc: (qc[1] >= 128, qc[0], qc[1]),
    )
    npairs = len(pairs)
    nchunks = -(-npairs // 128)
    npad = nchunks * 128

    # ---- cost model: D (two-level) vs A (tap-block, W-stationary) ----
    nzb = (Wp[:, :nblk_full * 128].reshape(C, nblk_full, 128) != 0.0).any(axis=2)
    Msb, keepb = [], []
    for i in range(nblk_full):
        idx = np.where(nzb[:, i])[0]
        if len(idx):
            keepb.append(i)
            Msb.append(int(idx[-1]) + 1)
    j_max_a = (max(keepb) // 4) if keepb else 0
    U = T_out + max(nq - 1, j_max_a)
    cost_d = nchunks * 4 * U + (nchunks + (1 if C > 128 else 0)) * T_out
    cost_a = (len(keepb) + sum(1 for m in Msb if m > 128)) * T_out
    mode = "D" if (cost_d < cost_a and C <= 256) else "A"

    if mode == "D":
        U = T_out + nq - 1
        wmax = float(np.abs(Wp).max())
        wexp = int(np.floor(np.log2(0.25 / wmax))) if wmax > 0 else 0
        scale = np.float32(2.0 ** wexp)
        # wq[r, k, p] = Wp[c, 512q + 128k + r] * scale
        wq = np.zeros((128, 4, npad), dtype=np.float16)
        ee = np.zeros((128, nchunks * C), dtype=np.float16)
        runs = [[] for _ in range(nchunks)]
        has_b = [False] * nchunks
        for p, (q, c) in enumerate(pairs + [(0, 0)] * (npad - npairs)):
            m, r = divmod(p, 128)
            if p < npairs:
                wq[:, :, p] = (Wp[c, 512 * q: 512 * (q + 1)] * scale).reshape(4, 128).T
                ee[r, m * C + c] = 1.0
                if c >= 128:
                    has_b[m] = True
            if runs[m] and runs[m][-1][2] == q:
                runs[m][-1] = (runs[m][-1][0], r + 1, q)
            else:
                runs[m].append((r, r + 1, q))
        wq = np.ascontiguousarray(wq.reshape(128, 4 * npad))
        xdt = np.float16
        meta = (nq, nchunks, tuple(
            tuple(rr) for rr in ((tuple(t) for t in runs[m]) for m in range(nchunks))
        ), tuple(has_b))
        wt, keep, Ms, offs = wq, None, None, None
        ew = ee
    else:
        wexp = 0
        # ragged 128-tap blocks, desc active-prefix order (A path)
        keep = np.asarray(keepb, dtype=np.int64)
        Ms = np.asarray(Msb, dtype=np.int64)
        order = np.argsort(-Ms, kind="stable")
        keep = keep[order]
        Ms = Ms[order]
        wblk = Wp[:, :nblk_full * 128].reshape(C, nblk_full, 128)
        wt = np.ascontiguousarray(
            np.concatenate([wblk[:m, i, :].T for i, m in zip(keep, Ms)], axis=1)
        )
        offs = np.concatenate([[0], np.cumsum(Ms)]).tolist()
        keep = keep.tolist()
        Ms = Ms.tolist()
        U = T_out + max(keep) // 4
        xdt = np.float32
        meta = None
        ew = None

    xpad_len = 512 * U
    assert xpad_len >= pad + T, (xpad_len, pad + T)
    xp = np.zeros((B, xpad_len), dtype=xdt)
    xp[:, pad:pad + T] = x.astype(xdt)
    # xt[b, r, k*U + u] = xp[b, 512u + 128k + r]
    xt = np.ascontiguousarray(
        xp.reshape(B, U, 4, 128).transpose(0, 3, 2, 1).reshape(B, 128, 4 * U)
    )
    return dict(
        xt=xt, wt=wt, ew=ew, keep=keep, Ms=Ms, offs=offs, C=C, U=U,
        T_out=T_out, nbins=nbins, mode=mode, wexp=wexp, meta=meta,
    )


def _tiles(total, step):
    return [(t0, min(step, total - t0)) for t0 in range(0, total, step)]


def _build_program_d(b_per, C, U, T_out, nq, nchunks, runs, has_b):
    import concourse.mybir as mybir
    import concourse.tile as tile
    from concourse import bacc

    f32 = mybir.dt.float32
    f16 = mybir.dt.float16
    npad = nchunks * 128
    cb = C - 128 if C > 128 else 0
    u_tiles = _tiles(U, 512)
    t_tiles = _tiles(T_out, 512)
    b_chunks = [m for m in range(nchunks) if has_b[m]]

    nc = bacc.Bacc(
        "TRN2",
        target_bir_lowering=False,
        debug=False,
        enable_asserts=True,
        num_devices=N_CORES,
    )
    xt_d = nc.dram_tensor("xt", [b_per, 128, 4 * U], f16, kind="ExternalInput").ap()
    wt_d = nc.dram_tensor("wt", [128, 4 * npad], f16, kind="ExternalInput").ap()
    ew_d = nc.dram_tensor("ew", [128, nchunks * C], f16, kind="ExternalInput").ap()
    out_d = nc.dram_tensor("out", [b_per, C, T_out], f32, kind="ExternalOutput").ap()

    with tile.TileContext(nc) as tc:
        with (
            tc.tile_pool(name="wpool", bufs=1) as wpool,
            tc.tile_pool(name="xpool", bufs=2) as xpool,
            tc.tile_pool(name="gspool", bufs=2) as gspool,
            tc.tile_pool(name="gtpool", bufs=3) as gtpool,
            tc.tile_pool(name="evpool", bufs=3) as evpool,
            tc.tile_pool(name="ps1pool", bufs=2, space="PSUM") as ps1pool,
            tc.tile_pool(name="ps2pool", bufs=2, space="PSUM") as ps2pool,
        ):
            wsb = wpool.tile([128, 4 * npad], f16)
            esb = wpool.tile([128, nchunks * C], f16)
            wk = wsb.rearrange("r (k p) -> r k p", k=4)

            def dma_x_chunk(xb_tile, b, u0, u1):
                src = xt_d[b].rearrange("r (k u) -> r k u", k=4)
                dst = xb_tile.rearrange("r (k u) -> r k u", k=4)
                nc.sync.dma_start(out=dst[:, :, u0:u1], in_=src[:, :, u0:u1])

            # weights + selection matrices, chunked in consumption order
            for k in range(4):
                nc.sync.dma_start(
                    out=wk[:, k, :], in_=wt_d.rearrange("r (k p) -> r k p", k=4)[:, k, :]
                )
            nc.sync.dma_start(out=esb[:], in_=ew_d[:])

            def level1(xb, gs, u0, nu):
                for m in range(nchunks):
                    ps = ps1pool.tile([128, 512], f32, tag="ps1")
                    for k in range(4):
                        nc.tensor.matmul(
                            ps[:, :nu],
                            lhsT=wk[:, k, m * 128:(m + 1) * 128],
                            rhs=xb[:, k * U + u0: k * U + u0 + nu],
                            start=(k == 0),
                            stop=(k == 3),
                        )
                    # engines need 32-aligned partition bases, so evict the
                    # full chunk to a staging tile, then apply the per-run
                    # q-shift with SBUF->SBUF DMAs (partition-arbitrary)
                    gt = gtpool.tile([128, 512], f16, tag="gt")
                    nc.vector.tensor_copy(gt[:, :nu], ps[:, :nu])
                    for (r0, r1, q) in runs[m]:
                        a = max(0, u0 - q)
                        bcol = min(T_out, u0 + nu - q)
                        if bcol > a:
                            nc.sync.dma_start(
                                out=gs[r0:r1, m * T_out + a: m * T_out + bcol],
                                in_=gt[r0:r1, a + q - u0: bcol + q - u0],
                            )

            def level2(gs, b, t0, nt):
                pa = ps2pool.tile([128, 512], f32, tag="pa")
                if cb:
                    pb = ps2pool.tile([128, 512], f32, tag="pb")
                for m in range(nchunks):
                    rhs = gs[:, m * T_out + t0: m * T_out + t0 + nt]
                    nc.tensor.matmul(
                        pa[:min(C, 128), :nt],
                        lhsT=esb[:, m * C: m * C + min(C, 128)],
                        rhs=rhs,
                        start=(m == 0),
                        stop=(m == nchunks - 1),
                    )
                    if cb and has_b[m]:
                        nc.tensor.matmul(
                            pb[:cb, :nt],
                            lhsT=esb[:, m * C + 128: m * C + C],
                            rhs=rhs,
                            start=(m == b_chunks[0]),
                            stop=(m == b_chunks[-1]),
                        )
                eva = evpool.tile([128, 512], f32, tag="eva")
                nc.vector.tensor_copy(eva[:min(C, 128), :nt], pa[:min(C, 128), :nt])
                nc.sync.dma_start(
                    out=out_d[b, 0:min(C, 128), t0:t0 + nt],
                    in_=eva[:min(C, 128), :nt],
                )
                if cb:
                    evb = evpool.tile([128, 512], f32, tag="evb")
                    nc.vector.tensor_copy(evb[:cb, :nt], pb[:cb, :nt])
                    nc.sync.dma_start(
                        out=out_d[b, 128:C, t0:t0 + nt], in_=evb[:cb, :nt]
                    )

            for b in range(b_per):
                xb = xpool.tile([128, 4 * U], f16, tag="xb", name=f"xb{b}")
                for (u0, nu) in u_tiles:
                    dma_x_chunk(xb, b, u0, u0 + nu)
                gs = gspool.tile([128, nchunks * T_out], f16, tag="gs")
                # interleave: L2 for t-tile i runs once u-tiles 0..i+1 evicted
                done_u = 0
                emitted_t = 0
                for i, (u0, nu) in enumerate(u_tiles):
                    level1(xb, gs, u0, nu)
                    done_u = u0 + nu
                    while emitted_t < len(t_tiles):
                        t0, nt = t_tiles[emitted_t]
                        if t0 + nt + (nq - 1) <= done_u or i == len(u_tiles) - 1:
                            level2(gs, b, t0, nt)
                            emitted_t += 1
                        else:
                            break
    nc.compile()
    return nc


def _build_program_a(b_per, C, U, T_out, keep, Ms, offs):
    """A formulation (fallback for dense kernels): weights stationary,
    x moving, fp32r; out[b, c, t]."""
    import concourse.mybir as mybir
    import concourse.tile as tile
    from concourse import bacc

    f32 = mybir.dt.float32
    f32r = mybir.dt.float32r
    nblk = len(keep)
    sum_m = offs[-1]
    mb_max = max(max(Ms) - 128, 0)
    nts = [512] * (T_out // 512) + ([T_out % 512] if T_out % 512 else [])
    a_ps = list(range(nblk))
    b_ps = [p for p in a_ps if Ms[p] > 128]
    j_max = max(keep) // 4
    w_budgets = [192, 256, 512] + [704] * nblk
    w_chunks = []
    p0 = 0
    while p0 < nblk:
        budget = w_budgets[len(w_chunks)]
        p1 = p0 + 1
        while p1 < nblk and offs[p1 + 1] - offs[p0] <= budget:
            p1 += 1
        w_chunks.append((p0, p1))
        p0 = p1
    x_stops = []
    t0 = 0
    for nt in nts:
        x_stops.append(min(t0 + nt + j_max + 1, U))
        t0 += nt
    x_stops[-1] = U
    x_chunks = []
    u0 = 0
    for u1 in x_stops:
        if u1 > u0:
            x_chunks.append((u0, u1))
            u0 = u1

    nc = bacc.Bacc(
        "TRN2",
        target_bir_lowering=False,
        debug=False,
        enable_asserts=True,
        num_devices=N_CORES,
    )
    xt_d = nc.dram_tensor("xt", [b_per, 128, 4 * U], f32r, kind="ExternalInput").ap()
    wt_d = nc.dram_tensor("wt", [128, sum_m], f32r, kind="ExternalInput").ap()
    out_d = nc.dram_tensor("out", [b_per, C, T_out], f32, kind="ExternalOutput").ap()

    with tile.TileContext(nc) as tc:
        with (
            tc.tile_pool(name="wpool", bufs=1) as wpool,
            tc.tile_pool(name="xpool", bufs=2) as xpool,
            tc.tile_pool(name="evpool", bufs=3) as evpool,
            tc.tile_pool(name="pspool", bufs=2, space="PSUM") as pspool,
        ):
            wsb = wpool.tile([128, sum_m], f32r)

            def dma_x_chunk(xb_tile, b, u0, u1, ks):
                src = xt_d[b].rearrange("r (k u) -> r k u", k=4)
                dst = xb_tile.rearrange("r (k u) -> r k u", k=4)
                nc.sync.dma_start(
                    out=dst[:, ks[0]:ks[-1] + 1, u0:u1],
                    in_=src[:, ks[0]:ks[-1] + 1, u0:u1],
                )

            xb0 = xpool.tile([128, 4 * U], f32r, tag="xb", name="xb0")
            k_first = []
            for p in a_ps:
                k = keep[p] % 4
                if k not in k_first:
                    k_first.append(k)
            x_emits = [(x_chunks[0], (k,)) for k in k_first]
            x_emits += [(ch, (0, 1, 2, 3)) for ch in x_chunks[1:]]
            emits = []
            for i in range(max(len(x_emits), len(w_chunks))):
                if i < len(x_emits):
                    emits.append(("x", x_emits[i]))
                if i < len(w_chunks):
                    emits.append(("w", w_chunks[i]))
            for kind, args in emits:
                if kind == "x":
                    (u0, u1), ks = args
                    dma_x_chunk(xb0, 0, u0, u1, ks)
                else:
                    a0, a1 = args
                    nc.sync.dma_start(
                        out=wsb[:, offs[a0]:offs[a1]],
                        in_=wt_d[:, offs[a0]:offs[a1]],
                    )

            for b in range(b_per):
                if b == 0:
                    xb = xb0
                else:
                    xb = xpool.tile([128, 4 * U], f32r, tag="xb", name=f"xb{b}")
                    nc.sync.dma_start(out=xb[:], in_=xt_d[b])
                t0 = 0
                for nt in nts:
                    pa = pspool.tile([128, 512], f32, tag="pa")
                    if mb_max:
                        pb = pspool.tile([128, 512], f32, tag="pb")
                    for pos, p in enumerate(a_ps):
                        m = Ms[p]
                        j, k = divmod(keep[p], 4)
                        rhs = xb[:, k * U + t0 + j: k * U + t0 + j + nt]
                        ma = min(m, 128)
                        nc.tensor.matmul(
                            pa[:ma, :nt],
                            lhsT=wsb[:, offs[p]: offs[p] + ma],
                            rhs=rhs,
                            start=(pos == 0),
                            stop=(pos == len(a_ps) - 1),
                        )
                        if m > 128:
                            nc.tensor.matmul(
                                pb[:m - 128, :nt],
                                lhsT=wsb[:, offs[p] + 128: offs[p] + m],
                                rhs=rhs,
                                start=(p == b_ps[0]),
                                stop=(p == b_ps[-1]),
                            )
                    ma1 = min(Ms[a_ps[0]], 128)
                    eva = evpool.tile([128, 512], f32, tag="eva")
                    nc.vector.tensor_copy(eva[:ma1, :nt], pa[:ma1, :nt])
                    nc.sync.dma_start(
                        out=out_d[b, 0:ma1, t0:t0 + nt], in_=eva[:ma1, :nt]
                    )
                    if mb_max:
                        evb = evpool.tile([128, 512], f32, tag="evb")
                        nc.vector.tensor_copy(evb[:mb_max, :nt], pb[:mb_max, :nt])
                        nc.sync.dma_start(
                            out=out_d[b, 128:128 + mb_max, t0:t0 + nt],
                            in_=evb[:mb_max, :nt],
                        )
                    t0 += nt
    nc.compile()
    return nc


def _ensure_trace_shims():
    """If run_bass_kernel_spmd is invoked with tracing enabled (e.g. via
    BASS_TRACE=1) it imports antenv.axon_hooks and uploads artifacts to a
    bucket; neither exists in a bare container.  Register a working NTFF
    hook (ctypes into the axon .so) and a no-op uploader so the trace path
    degrades gracefully instead of crashing."""
    import sys

    try:
        import antenv.axon_hooks  # noqa: F401
    except ImportError:
        import contextlib
        import ctypes
        import types

        hook = None
        try:
            lib = ctypes.CDLL("/opt/axon/libaxon_pjrt.so")
            if hasattr(lib, "axon_start_nrt_profile"):
                lib.axon_start_nrt_profile.argtypes = [
                    ctypes.POINTER(ctypes.c_int64),
                    ctypes.c_size_t,
                ]
                lib.axon_start_nrt_profile.restype = ctypes.c_int64
                lib.axon_stop_nrt_profile.argtypes = [ctypes.c_char_p]
                lib.axon_stop_nrt_profile.restype = ctypes.c_int64

                @contextlib.contextmanager
                def _hook(output_dir, device_ids):
                    import jax

                    jax.devices()
                    if device_ids:
                        ids = (ctypes.c_int64 * len(device_ids))(*device_ids)
                        rc = lib.axon_start_nrt_profile(ids, len(device_ids))
                    else:
                        rc = lib.axon_start_nrt_profile(None, 0)
                    if rc != 0:
                        raise RuntimeError(f"axon_start_nrt_profile rc={rc}")
                    try:
                        yield
                    finally:
                        lib.axon_stop_nrt_profile(str(output_dir).encode())

                hook = _hook
        except OSError:
            pass
        mod = types.ModuleType("antenv.axon_hooks")
        mod.get_axon_ntff_profile_hook = lambda: hook
        mod.set_axon_ntff_profile_hook = lambda h: None
        sys.modules["antenv.axon_hooks"] = mod

    try:
        import concourse.bass_utils as _bu

        _orig_upload = _bu.upload_artifacts

        def _safe_upload(tmpdir):
            try:
                return _orig_upload(tmpdir)
            except Exception:
                return "local://unavailable"

        if not getattr(_bu, "_safe_upload_installed", False):
            _bu.upload_artifacts = _safe_upload
            _bu._safe_upload_installed = True
    except Exception:
        pass


def kernel(x, kernels):
    _ensure_trace_shims()
    from concourse.bass_utils import run_bass_kernel_spmd

    hp = _host_prep(x, kernels)
    xt, mode, wexp = hp["xt"], hp["mode"], hp["wexp"]
    C, U, T_out, nbins = hp["C"], hp["U"], hp["T_out"], hp["nbins"]
    B = xt.shape[0]
    assert B % N_CORES == 0
    b_per = B // N_CORES

    if mode == "D":
        nq, nchunks, runs, has_b = hp["meta"]
        runs = [list(r) for r in runs]
        key = ("D", b_per, C, U, T_out, nq, nchunks,
               tuple(tuple(t) for rr in runs for t in rr), has_b)
        if key not in _prog_cache:
            _prog_cache[key] = _build_program_d(
                b_per, C, U, T_out, nq, nchunks, runs, list(has_b)
            )
        nc = _prog_cache[key]
        in_maps = [
            {"xt": xt[c * b_per:(c + 1) * b_per], "wt": hp["wt"], "ew": hp["ew"]}
            for c in range(N_CORES)
        ]
    else:
        keep, Ms, offs = hp["keep"], hp["Ms"], hp["offs"]
        key = ("A", b_per, C, U, T_out, tuple(keep), tuple(Ms))
        if key not in _prog_cache:
            _prog_cache[key] = _build_program_a(b_per, C, U, T_out, keep, Ms, offs)
        nc = _prog_cache[key]
        in_maps = [
            {"xt": xt[c * b_per:(c + 1) * b_per], "wt": hp["wt"]}
            for c in range(N_CORES)
        ]

    res = run_bass_kernel_spmd(nc, in_maps, list(range(N_CORES)))
    parts = [res.results[c]["out"] for c in range(N_CORES)]
    out = np.concatenate(parts, axis=0)  # (B, C, T_out)
    if wexp:
        out = out * np.float32(2.0 ** -wexp)
    return np.ascontiguousarray(
        out.reshape(B, nbins, 2, T_out).transpose(0, 2, 1, 3)
    )


# revision 11
# speedup vs baseline: 1.7386x; 1.7386x over previous
"""CQT (constant-Q transform) kernel for Trainium2, 8 NeuronCores.

Math: out[b, c, t] = sum_l W[c, l] * x_pad[b, t*HOP + l]   (strided conv,
HOP=512, L=11339 taps, C=168 channels = 84 bins x re/im), then reshaped to
(B, 2, n_bins, T_out).

Strategy (two-level factorization, data-parallel over batch):
  - Write l = 512q + s.  Level 1 contracts s: with the polyphase matrix
    Y[s, u] = xp[512u + s], compute G[(c,q), u] = sum_s W[c,512q+s] Y[s,u]
    for the ~815 ACTIVE (channel, hop-block) pairs only (CQT kernels are
    ragged: bin k has ~11339*2^(-k/12) centered taps, so sum_c ceil(l_c/512)
    ~= 815 of 168*23 possible pairs).  Packed into ceil(815/128) = 7 dense
    chunks of 128 pairs -> 28 matmuls per u-sweep at N=512, ~91% PE
    utilization (vs 29% for the naive 128-tap-block decomposition).
  - Shift on evict: PSUM->SBUF copies write Gs[p, t] = G[p, t+q(p)] (pairs
    sharing q form contiguous runs, so each run is one affine copy),
    converting fp32->fp16.
  - Level 2 contracts q: out[c, t] = sum_p E[p, c] Gs[p, t] with one-hot
    E per chunk -- 7+1 matmuls of N=512 per t-tile into the usual
    [C, t] PSUM layout.
  - fp16 x and 2^wexp-scaled fp16 W (undone on host); fp32 PSUM.
  - A formulation (weights stationary, x moving, fp32r) kept as fallback
    if the kernel tensor has no zero raggedness.
"""

import numpy as np

HOP = 512
N_CORES = 8

_prog_cache: dict = {}


def _host_prep(x, kernels):
    x = np.ascontiguousarray(np.asarray(x, dtype=np.float32))
    kernels = np.ascontiguousarray(np.asarray(kernels, dtype=np.float32))
    B, T = x.shape
    nbins, two, Lmax = kernels.shape
    assert two == 2
    C = 2 * nbins
    pad = Lmax // 2
    T_out = (T + 2 * pad - Lmax) // HOP + 1

    nblk_full = -(-Lmax // 128)
    nq = -(-(nblk_full * 128) // 512)
    Wp = np.zeros((C, nq * 512), dtype=np.float32)
    Wp[:, :Lmax] = kernels.reshape(C, Lmax)

    # ---- active (q, c) pairs for the D formulation ----
    seg_nz = (Wp.reshape(C, nq, 512) != 0.0).any(axis=2)  # [C, nq]
    # cluster c>=128 pairs at the end so they occupy the fewest chunks
    # (each chunk holding such pairs costs an extra matmul per t-tile)
    pairs = sorted(
        ((q, c) for q in range(nq) for c in range(C) if seg_nz[c, q]),
        key=lambda qc: (qc[1] >= 128, qc[0], qc[1]),
    )
    npairs = len(pairs)
    nchunks = -(-npairs // 128)
    npad = nchunks * 128

    # ---- cost model: D (two-level) vs A (tap-block, W-stationary) ----
    nzb = (Wp[:, :nblk_full * 128].reshape(C, nblk_full, 128) != 0.0).any(axis=2)
    Msb, keepb = [], []
    for i in range(nblk_full):
        idx = np.where(nzb[:, i])[0]
        if len(idx):
            keepb.append(i)
            Msb.append(int(idx[-1]) + 1)
    j_max_a = (max(keepb) // 4) if keepb else 0
    U = T_out + max(nq - 1, j_max_a)
    cost_d = nchunks * 4 * U + (nchunks + (1 if C > 128 else 0)) * T_out
    cost_a = (len(keepb) + sum(1 for m in Msb if m > 128)) * T_out
    mode = "D" if (cost_d < cost_a and C <= 256) else "A"

    if mode == "D":
        U = T_out + nq - 1
        wmax = float(np.abs(Wp).max())
        wexp = int(np.floor(np.log2(0.25 / wmax))) if wmax > 0 else 0
        scale = np.float32(2.0 ** wexp)
        # wq[r, k, p] = Wp[c, 512q + 128k + r] * scale
        wq = np.zeros((128, 4, npad), dtype=np.float16)
        ee = np.zeros((128, nchunks * C), dtype=np.float16)
        runs = [[] for _ in range(nchunks)]
        has_b = [False] * nchunks
        for p, (q, c) in enumerate(pairs + [(0, 0)] * (npad - npairs)):
            m, r = divmod(p, 128)
            if p < npairs:
                wq[:, :, p] = (Wp[c, 512 * q: 512 * (q + 1)] * scale).reshape(4, 128).T
                ee[r, m * C + c] = 1.0
                if c >= 128:
                    has_b[m] = True
            if runs[m] and runs[m][-1][2] == q:
                runs[m][-1] = (runs[m][-1][0], r + 1, q)
            else:
                runs[m].append((r, r + 1, q))
        wq = np.ascontiguousarray(wq.reshape(128, 4 * npad))
        xdt = np.float16
        meta = (nq, nchunks, tuple(
            tuple(rr) for rr in ((tuple(t) for t in runs[m]) for m in range(nchunks))
        ), tuple(has_b))
        wt, keep, Ms, offs = wq, None, None, None
        ew = ee
    else:
        wexp = 0
        # ragged 128-tap blocks, desc active-prefix order (A path)
        keep = np.asarray(keepb, dtype=np.int64)
        Ms = np.asarray(Msb, dtype=np.int64)
        order = np.argsort(-Ms, kind="stable")
        keep = keep[order]
        Ms = Ms[order]
        wblk = Wp[:, :nblk_full * 128].reshape(C, nblk_full, 128)
        wt = np.ascontiguousarray(
            np.concatenate([wblk[:m, i, :].T for i, m in zip(keep, Ms)], axis=1)
        )
        offs = np.concatenate([[0], np.cumsum(Ms)]).tolist()
        keep = keep.tolist()
        Ms = Ms.tolist()
        U = T_out + max(keep) // 4
        xdt = np.float32
        meta = None
        ew = None

    xpad_len = 512 * U
    assert xpad_len >= pad + T, (xpad_len, pad + T)
    xp = np.zeros((B, xpad_len), dtype=xdt)
    xp[:, pad:pad + T] = x.astype(xdt)
    # xt[b, r, k*U + u] = xp[b, 512u + 128k + r]
    xt = np.ascontiguousarray(
        xp.reshape(B, U, 4, 128).transpose(0, 3, 2, 1).reshape(B, 128, 4 * U)
    )
    return dict(
        xt=xt, wt=wt, ew=ew, keep=keep, Ms=Ms, offs=offs, C=C, U=U,
        T_out=T_out, nbins=nbins, mode=mode, wexp=wexp, meta=meta,
    )


def _tiles(total, step):
    return [(t0, min(step, total - t0)) for t0 in range(0, total, step)]


def _build_program_d(b_per, C, U, T_out, nq, nchunks, runs, has_b):
    import concourse.mybir as mybir
    import concourse.tile as tile
    from concourse import bacc

    f32 = mybir.dt.float32
    f16 = mybir.dt.float16
    npad = nchunks * 128
    cb = C - 128 if C > 128 else 0
    u_tiles = _tiles(U, 512)
    t_tiles = _tiles(T_out, 512)
    b_chunks = [m for m in range(nchunks) if has_b[m]]

    nc = bacc.Bacc(
        "TRN2",
        target_bir_lowering=False,
        debug=False,
        enable_asserts=True,
        num_devices=N_CORES,
    )
    xt_d = nc.dram_tensor("xt", [b_per, 128, 4 * U], f16, kind="ExternalInput").ap()
    wt_d = nc.dram_tensor("wt", [128, 4 * npad], f16, kind="ExternalInput").ap()
    ew_d = nc.dram_tensor("ew", [128, nchunks * C], f16, kind="ExternalInput").ap()
    out_d = nc.dram_tensor("out", [b_per, C, T_out], f32, kind="ExternalOutput").ap()

    with tile.TileContext(nc) as tc:
        with (
            tc.tile_pool(name="wpool", bufs=1) as wpool,
            tc.tile_pool(name="xpool", bufs=2) as xpool,
            tc.tile_pool(name="gspool", bufs=2) as gspool,
            tc.tile_pool(name="gtpool", bufs=2) as gtpool,
            tc.tile_pool(name="evpool", bufs=3) as evpool,
            tc.tile_pool(name="ps1pool", bufs=2, space="PSUM") as ps1pool,
            tc.tile_pool(name="ps2pool", bufs=2, space="PSUM") as ps2pool,
        ):
            wsb = wpool.tile([128, 4 * npad], f16)
            esb = wpool.tile([128, nchunks * C], f16)
            wk = wsb.rearrange("r (k p) -> r k p", k=4)

            def dma_x_chunk(xb_tile, b, u0, u1):
                src = xt_d[b].rearrange("r (k u) -> r k u", k=4)
                dst = xb_tile.rearrange("r (k u) -> r k u", k=4)
                nc.sync.dma_start(out=dst[:, :, u0:u1], in_=src[:, :, u0:u1])

            # weights + selection matrices, chunked in consumption order
            for k in range(4):
                nc.sync.dma_start(
                    out=wk[:, k, :], in_=wt_d.rearrange("r (k p) -> r k p", k=4)[:, k, :]
                )
            nc.sync.dma_start(out=esb[:], in_=ew_d[:])

            dma_engs = [nc.sync, nc.scalar, nc.gpsimd]
            rr_state = [0]

            def level1_chunk(xb, gs, m):
                # compute chunk m's G over the full U, staged in fp16, then
                # apply the per-run q-shift with one big SBUF->SBUF DMA per
                # run (engines need 32-aligned partition bases; DMA doesn't),
                # round-robined across queues so they run in parallel
                gt = gtpool.tile([128, U], f16, tag="gt")
                for (u0, nu) in u_tiles:
                    ps = ps1pool.tile([128, 512], f32, tag="ps1")
                    for k in range(4):
                        nc.tensor.matmul(
                            ps[:, :nu],
                            lhsT=wk[:, k, m * 128:(m + 1) * 128],
                            rhs=xb[:, k * U + u0: k * U + u0 + nu],
                            start=(k == 0),
                            stop=(k == 3),
                        )
                    nc.vector.tensor_copy(gt[:, u0:u0 + nu], ps[:, :nu])
                for (r0, r1, q) in runs[m]:
                    eng = dma_engs[rr_state[0] % len(dma_engs)]
                    rr_state[0] += 1
                    eng.dma_start(
                        out=gs[r0:r1, m * T_out: (m + 1) * T_out],
                        in_=gt[r0:r1, q: q + T_out],
                    )

            def level2(gs, b, t0, nt):
                pa = ps2pool.tile([128, 512], f32, tag="pa")
                if cb:
                    pb = ps2pool.tile([128, 512], f32, tag="pb")
                for m in range(nchunks):
                    rhs = gs[:, m * T_out + t0: m * T_out + t0 + nt]
                    nc.tensor.matmul(
                        pa[:min(C, 128), :nt],
                        lhsT=esb[:, m * C: m * C + min(C, 128)],
                        rhs=rhs,
                        start=(m == 0),
                        stop=(m == nchunks - 1),
                    )
                    if cb and has_b[m]:
                        nc.tensor.matmul(
                            pb[:cb, :nt],
                            lhsT=esb[:, m * C + 128: m * C + C],
                            rhs=rhs,
                            start=(m == b_chunks[0]),
                            stop=(m == b_chunks[-1]),
                        )
                eva = evpool.tile([128, 512], f32, tag="eva")
                nc.vector.tensor_copy(eva[:min(C, 128), :nt], pa[:min(C, 128), :nt])
                nc.sync.dma_start(
                    out=out_d[b, 0:min(C, 128), t0:t0 + nt],
                    in_=eva[:min(C, 128), :nt],
                )
                if cb:
                    evb = evpool.tile([128, 512], f32, tag="evb")
                    nc.vector.tensor_copy(evb[:cb, :nt], pb[:cb, :nt])
                    nc.sync.dma_start(
                        out=out_d[b, 128:C, t0:t0 + nt], in_=evb[:cb, :nt]
                    )

            for b in range(b_per):
                xb = xpool.tile([128, 4 * U], f16, tag="xb", name=f"xb{b}")
                for (u0, nu) in u_tiles:
                    dma_x_chunk(xb, b, u0, u0 + nu)
                gs = gspool.tile([128, nchunks * T_out], f16, tag="gs")
                for m in range(nchunks):
                    level1_chunk(xb, gs, m)
                for (t0, nt) in t_tiles:
                    level2(gs, b, t0, nt)
    nc.compile()
    return nc


def _build_program_a(b_per, C, U, T_out, keep, Ms, offs):
    """A formulation (fallback for dense kernels): weights stationary,
    x moving, fp32r; out[b, c, t]."""
    import concourse.mybir as mybir
    import concourse.tile as tile
    from concourse import bacc

    f32 = mybir.dt.float32
    f32r = mybir.dt.float32r
    nblk = len(keep)
    sum_m = offs[-1]
    mb_max = max(max(Ms) - 128, 0)
    nts = [512] * (T_out // 512) + ([T_out % 512] if T_out % 512 else [])
    a_ps = list(range(nblk))
    b_ps = [p for p in a_ps if Ms[p] > 128]
    j_max = max(keep) // 4
    w_budgets = [192, 256, 512] + [704] * nblk
    w_chunks = []
    p0 = 0
    while p0 < nblk:
        budget = w_budgets[len(w_chunks)]
        p1 = p0 + 1
        while p1 < nblk and offs[p1 + 1] - offs[p0] <= budget:
            p1 += 1
        w_chunks.append((p0, p1))
        p0 = p1
    x_stops = []
    t0 = 0
    for nt in nts:
        x_stops.append(min(t0 + nt + j_max + 1, U))
        t0 += nt
    x_stops[-1] = U
    x_chunks = []
    u0 = 0
    for u1 in x_stops:
        if u1 > u0:
            x_chunks.append((u0, u1))
            u0 = u1

    nc = bacc.Bacc(
        "TRN2",
        target_bir_lowering=False,
        debug=False,
        enable_asserts=True,
        num_devices=N_CORES,
    )
    xt_d = nc.dram_tensor("xt", [b_per, 128, 4 * U], f32r, kind="ExternalInput").ap()
    wt_d = nc.dram_tensor("wt", [128, sum_m], f32r, kind="ExternalInput").ap()
    out_d = nc.dram_tensor("out", [b_per, C, T_out], f32, kind="ExternalOutput").ap()

    with tile.TileContext(nc) as tc:
        with (
            tc.tile_pool(name="wpool", bufs=1) as wpool,
            tc.tile_pool(name="xpool", bufs=2) as xpool,
            tc.tile_pool(name="evpool", bufs=3) as evpool,
            tc.tile_pool(name="pspool", bufs=2, space="PSUM") as pspool,
        ):
            wsb = wpool.tile([128, sum_m], f32r)

            def dma_x_chunk(xb_tile, b, u0, u1, ks):
                src = xt_d[b].rearrange("r (k u) -> r k u", k=4)
                dst = xb_tile.rearrange("r (k u) -> r k u", k=4)
                nc.sync.dma_start(
                    out=dst[:, ks[0]:ks[-1] + 1, u0:u1],
                    in_=src[:, ks[0]:ks[-1] + 1, u0:u1],
                )

            xb0 = xpool.tile([128, 4 * U], f32r, tag="xb", name="xb0")
            k_first = []
            for p in a_ps:
                k = keep[p] % 4
                if k not in k_first:
                    k_first.append(k)
            x_emits = [(x_chunks[0], (k,)) for k in k_first]
            x_emits += [(ch, (0, 1, 2, 3)) for ch in x_chunks[1:]]
            emits = []
            for i in range(max(len(x_emits), len(w_chunks))):
                if i < len(x_emits):
                    emits.append(("x", x_emits[i]))
                if i < len(w_chunks):
                    emits.append(("w", w_chunks[i]))
            for kind, args in emits:
                if kind == "x":
                    (u0, u1), ks = args
                    dma_x_chunk(xb0, 0, u0, u1, ks)
                else:
                    a0, a1 = args
                    nc.sync.dma_start(
                        out=wsb[:, offs[a0]:offs[a1]],
                        in_=wt_d[:, offs[a0]:offs[a1]],
                    )

            for b in range(b_per):
                if b == 0:
                    xb = xb0
                else:
                    xb = xpool.tile([128, 4 * U], f32r, tag="xb", name=f"xb{b}")
                    nc.sync.dma_start(out=xb[:], in_=xt_d[b])
                t0 = 0
                for nt in nts:
                    pa = pspool.tile([128, 512], f32, tag="pa")
                    if mb_max:
                        pb = pspool.tile([128, 512], f32, tag="pb")
                    for pos, p in enumerate(a_ps):
                        m = Ms[p]
                        j, k = divmod(keep[p], 4)
                        rhs = xb[:, k * U + t0 + j: k * U + t0 + j + nt]
                        ma = min(m, 128)
                        nc.tensor.matmul(
                            pa[:ma, :nt],
                            lhsT=wsb[:, offs[p]: offs[p] + ma],
                            rhs=rhs,
                            start=(pos == 0),
                            stop=(pos == len(a_ps) - 1),
                        )
                        if m > 128:
                            nc.tensor.matmul(
                                pb[:m - 128, :nt],
                                lhsT=wsb[:, offs[p] + 128: offs[p] + m],
                                rhs=rhs,
                                start=(p == b_ps[0]),
                                stop=(p == b_ps[-1]),
                            )
                    ma1 = min(Ms[a_ps[0]], 128)
                    eva = evpool.tile([128, 512], f32, tag="eva")
                    nc.vector.tensor_copy(eva[:ma1, :nt], pa[:ma1, :nt])
                    nc.sync.dma_start(
                        out=out_d[b, 0:ma1, t0:t0 + nt], in_=eva[:ma1, :nt]
                    )
                    if mb_max:
                        evb = evpool.tile([128, 512], f32, tag="evb")
                        nc.vector.tensor_copy(evb[:mb_max, :nt], pb[:mb_max, :nt])
                        nc.sync.dma_start(
                            out=out_d[b, 128:128 + mb_max, t0:t0 + nt],
                            in_=evb[:mb_max, :nt],
                        )
                    t0 += nt
    nc.compile()
    return nc


def _ensure_trace_shims():
    """If run_bass_kernel_spmd is invoked with tracing enabled (e.g. via
    BASS_TRACE=1) it imports antenv.axon_hooks and uploads artifacts to a
    bucket; neither exists in a bare container.  Register a working NTFF
    hook (ctypes into the axon .so) and a no-op uploader so the trace path
    degrades gracefully instead of crashing."""
    import sys

    try:
        import antenv.axon_hooks  # noqa: F401
    except ImportError:
        import contextlib
        import ctypes
        import types

        hook = None
        try:
            lib = ctypes.CDLL("/opt/axon/libaxon_pjrt.so")
            if hasattr(lib, "axon_start_nrt_profile"):
                lib.axon_start_nrt_profile.argtypes = [
                    ctypes.POINTER(ctypes.c_int64),
                    ctypes.c_size_t,
                ]
                lib.axon_start_nrt_profile.restype = ctypes.c_int64
                lib.axon_stop_nrt_profile.argtypes = [ctypes.c_char_p]
                lib.axon_stop_nrt_profile.restype = ctypes.c_int64

                @contextlib.contextmanager
                def _hook(output_dir, device_ids):
                    import jax

                    jax.devices()
                    if device_ids:
                        ids = (ctypes.c_int64 * len(device_ids))(*device_ids)
                        rc = lib.axon_start_nrt_profile(ids, len(device_ids))
                    else:
                        rc = lib.axon_start_nrt_profile(None, 0)
                    if rc != 0:
                        raise RuntimeError(f"axon_start_nrt_profile rc={rc}")
                    try:
                        yield
                    finally:
                        lib.axon_stop_nrt_profile(str(output_dir).encode())

                hook = _hook
        except OSError:
            pass
        mod = types.ModuleType("antenv.axon_hooks")
        mod.get_axon_ntff_profile_hook = lambda: hook
        mod.set_axon_ntff_profile_hook = lambda h: None
        sys.modules["antenv.axon_hooks"] = mod

    try:
        import concourse.bass_utils as _bu

        _orig_upload = _bu.upload_artifacts

        def _safe_upload(tmpdir):
            try:
                return _orig_upload(tmpdir)
            except Exception:
                return "local://unavailable"

        if not getattr(_bu, "_safe_upload_installed", False):
            _bu.upload_artifacts = _safe_upload
            _bu._safe_upload_installed = True
    except Exception:
        pass


def kernel(x, kernels):
    _ensure_trace_shims()
    from concourse.bass_utils import run_bass_kernel_spmd

    hp = _host_prep(x, kernels)
    xt, mode, wexp = hp["xt"], hp["mode"], hp["wexp"]
    C, U, T_out, nbins = hp["C"], hp["U"], hp["T_out"], hp["nbins"]
    B = xt.shape[0]
    assert B % N_CORES == 0
    b_per = B // N_CORES

    if mode == "D":
        nq, nchunks, runs, has_b = hp["meta"]
        runs = [list(r) for r in runs]
        key = ("D", b_per, C, U, T_out, nq, nchunks,
               tuple(tuple(t) for rr in runs for t in rr), has_b)
        if key not in _prog_cache:
            _prog_cache[key] = _build_program_d(
                b_per, C, U, T_out, nq, nchunks, runs, list(has_b)
            )
        nc = _prog_cache[key]
        in_maps = [
            {"xt": xt[c * b_per:(c + 1) * b_per], "wt": hp["wt"], "ew": hp["ew"]}
            for c in range(N_CORES)
        ]
    else:
        keep, Ms, offs = hp["keep"], hp["Ms"], hp["offs"]
        key = ("A", b_per, C, U, T_out, tuple(keep), tuple(Ms))
        if key not in _prog_cache:
            _prog_cache[key] = _build_program_a(b_per, C, U, T_out, keep, Ms, offs)
        nc = _prog_cache[key]
        in_maps = [
            {"xt": xt[c * b_per:(c + 1) * b_per], "wt": hp["wt"]}
            for c in range(N_CORES)
        ]

    res = run_bass_kernel_spmd(nc, in_maps, list(range(N_CORES)))
    parts = [res.results[c]["out"] for c in range(N_CORES)]
    out = np.concatenate(parts, axis=0)  # (B, C, T_out)
    if wexp:
        out = out * np.float32(2.0 ** -wexp)
    return np.ascontiguousarray(
        out.reshape(B, nbins, 2, T_out).transpose(0, 2, 1, 3)
    )


# revision 14
# speedup vs baseline: 1.8088x; 1.0404x over previous
"""CQT (constant-Q transform) kernel for Trainium2, 8 NeuronCores.

Math: out[b, c, t] = sum_l W[c, l] * x_pad[b, t*HOP + l]   (strided conv,
HOP=512, L=11339 taps, C=168 channels = 84 bins x re/im), then reshaped to
(B, 2, n_bins, T_out).

Strategy (two-level factorization, data-parallel over batch):
  - Write l = 512q + s.  Level 1 contracts s: with the polyphase matrix
    Y[s, u] = xp[512u + s], compute G[(c,q), u] = sum_s W[c,512q+s] Y[s,u]
    for the ~815 ACTIVE (channel, hop-block) pairs only (CQT kernels are
    ragged: bin k has ~11339*2^(-k/12) centered taps, so sum_c ceil(l_c/512)
    ~= 815 of 168*23 possible pairs).  Packed into ceil(815/128) = 7 dense
    chunks of 128 pairs -> 28 matmuls per u-sweep at N=512, ~91% PE
    utilization (vs 29% for the naive 128-tap-block decomposition).
  - Shift on evict: PSUM->SBUF copies write Gs[p, t] = G[p, t+q(p)] (pairs
    sharing q form contiguous runs, so each run is one affine copy),
    converting fp32->fp16.
  - Level 2 contracts q: out[c, t] = sum_p E[p, c] Gs[p, t] with one-hot
    E per chunk -- 7+1 matmuls of N=512 per t-tile into the usual
    [C, t] PSUM layout.
  - fp16 x and 2^wexp-scaled fp16 W (undone on host); fp32 PSUM.
  - A formulation (weights stationary, x moving, fp32r) kept as fallback
    if the kernel tensor has no zero raggedness.
"""

import numpy as np

HOP = 512
N_CORES = 8

_prog_cache: dict = {}


def _host_prep(x, kernels):
    x = np.ascontiguousarray(np.asarray(x, dtype=np.float32))
    kernels = np.ascontiguousarray(np.asarray(kernels, dtype=np.float32))
    B, T = x.shape
    nbins, two, Lmax = kernels.shape
    assert two == 2
    C = 2 * nbins
    pad = Lmax // 2
    T_out = (T + 2 * pad - Lmax) // HOP + 1

    nblk_full = -(-Lmax // 128)
    nq = -(-(nblk_full * 128) // 512)
    Wp = np.zeros((C, nq * 512), dtype=np.float32)
    Wp[:, :Lmax] = kernels.reshape(C, Lmax)

    # ---- active (q, c) pairs for the D formulation ----
    seg_nz = (Wp.reshape(C, nq, 512) != 0.0).any(axis=2)  # [C, nq]
    # cluster c>=128 pairs at the end so they occupy the fewest chunks
    # (each chunk holding such pairs costs an extra matmul per t-tile)
    pairs = sorted(
        ((q, c) for q in range(nq) for c in range(C) if seg_nz[c, q]),
        key=lambda qc: (qc[1] >= 128, qc[0], qc[1]),
    )
    npairs = len(pairs)
    nchunks = -(-npairs // 128)
    npad = nchunks * 128

    # ---- cost model: D (two-level) vs A (tap-block, W-stationary) ----
    nzb = (Wp[:, :nblk_full * 128].reshape(C, nblk_full, 128) != 0.0).any(axis=2)
    Msb, keepb = [], []
    for i in range(nblk_full):
        idx = np.where(nzb[:, i])[0]
        if len(idx):
            keepb.append(i)
            Msb.append(int(idx[-1]) + 1)
    j_max_a = (max(keepb) // 4) if keepb else 0
    U = T_out + max(nq - 1, j_max_a)
    cost_d = nchunks * 4 * U + (nchunks + (1 if C > 128 else 0)) * T_out
    cost_a = (len(keepb) + sum(1 for m in Msb if m > 128)) * T_out
    mode = "D" if (cost_d < cost_a and C <= 256) else "A"

    if mode == "D":
        U = T_out + nq - 1
        wmax = float(np.abs(Wp).max())
        wexp = int(np.floor(np.log2(0.25 / wmax))) if wmax > 0 else 0
        scale = np.float32(2.0 ** wexp)
        # wq[r, k, p] = Wp[c, 512q + 128k + r] * scale
        wq = np.zeros((128, 4, npad), dtype=np.float16)
        ee = np.zeros((128, nchunks * C), dtype=np.float16)
        runs = [[] for _ in range(nchunks)]
        has_b = [False] * nchunks
        for p, (q, c) in enumerate(pairs + [(0, 0)] * (npad - npairs)):
            m, r = divmod(p, 128)
            if p < npairs:
                wq[:, :, p] = (Wp[c, 512 * q: 512 * (q + 1)] * scale).reshape(4, 128).T
                ee[r, m * C + c] = 2.0 ** -wexp  # folds the W-scale undo into L2
                if c >= 128:
                    has_b[m] = True
            if runs[m] and runs[m][-1][2] == q:
                runs[m][-1] = (runs[m][-1][0], r + 1, q)
            else:
                runs[m].append((r, r + 1, q))
        wq = np.ascontiguousarray(wq.reshape(128, 4 * npad))
        wexp = 0  # already undone via ee
        xdt = np.float16
        meta = (nq, nchunks, tuple(
            tuple(rr) for rr in ((tuple(t) for t in runs[m]) for m in range(nchunks))
        ), tuple(has_b))
        wt, keep, Ms, offs = wq, None, None, None
        ew = ee
    else:
        wexp = 0
        # ragged 128-tap blocks, desc active-prefix order (A path)
        keep = np.asarray(keepb, dtype=np.int64)
        Ms = np.asarray(Msb, dtype=np.int64)
        order = np.argsort(-Ms, kind="stable")
        keep = keep[order]
        Ms = Ms[order]
        wblk = Wp[:, :nblk_full * 128].reshape(C, nblk_full, 128)
        wt = np.ascontiguousarray(
            np.concatenate([wblk[:m, i, :].T for i, m in zip(keep, Ms)], axis=1)
        )
        offs = np.concatenate([[0], np.cumsum(Ms)]).tolist()
        keep = keep.tolist()
        Ms = Ms.tolist()
        U = T_out + max(keep) // 4
        xdt = np.float32
        meta = None
        ew = None

    xpad_len = 512 * U
    assert xpad_len >= pad + T, (xpad_len, pad + T)
    xp = np.zeros((B, xpad_len), dtype=xdt)
    xp[:, pad:pad + T] = x.astype(xdt)
    # xt[b, r, k*U + u] = xp[b, 512u + 128k + r]
    xt = np.ascontiguousarray(
        xp.reshape(B, U, 4, 128).transpose(0, 3, 2, 1).reshape(B, 128, 4 * U)
    )
    return dict(
        xt=xt, wt=wt, ew=ew, keep=keep, Ms=Ms, offs=offs, C=C, U=U,
        T_out=T_out, nbins=nbins, mode=mode, wexp=wexp, meta=meta,
    )


def _tiles(total, step):
    return [(t0, min(step, total - t0)) for t0 in range(0, total, step)]


def _build_program_d(b_per, C, U, T_out, nq, nchunks, runs, has_b):
    import concourse.mybir as mybir
    import concourse.tile as tile
    from concourse import bacc

    f32 = mybir.dt.float32
    f16 = mybir.dt.float16
    npad = nchunks * 128
    cb = C - 128 if C > 128 else 0
    u_tiles = _tiles(U, 512)
    t_tiles = _tiles(T_out, 512)
    b_chunks = [m for m in range(nchunks) if has_b[m]]

    nc = bacc.Bacc(
        "TRN2",
        target_bir_lowering=False,
        debug=False,
        enable_asserts=True,
        num_devices=N_CORES,
    )
    xt_d = nc.dram_tensor("xt", [b_per, 128, 4 * U], f16, kind="ExternalInput").ap()
    wt_d = nc.dram_tensor("wt", [128, 4 * npad], f16, kind="ExternalInput").ap()
    ew_d = nc.dram_tensor("ew", [128, nchunks * C], f16, kind="ExternalInput").ap()
    out_d = nc.dram_tensor("out", [b_per, C, T_out], f32, kind="ExternalOutput").ap()

    with tile.TileContext(nc) as tc:
        with (
            tc.tile_pool(name="wpool", bufs=1) as wpool,
            tc.tile_pool(name="xpool", bufs=2) as xpool,
            tc.tile_pool(name="gspool", bufs=2) as gspool,
            tc.tile_pool(name="gtpool", bufs=2) as gtpool,
            tc.tile_pool(name="evpool", bufs=3) as evpool,
            tc.tile_pool(name="ps1pool", bufs=2, space="PSUM") as ps1pool,
            tc.tile_pool(name="ps2pool", bufs=2, space="PSUM") as ps2pool,
        ):
            wsb = wpool.tile([128, 4 * npad], f16)
            esb = wpool.tile([128, nchunks * C], f16)
            wk = wsb.rearrange("r (k p) -> r k p", k=4)

            def dma_x_chunk(xb_tile, b, u0, u1):
                src = xt_d[b].rearrange("r (k u) -> r k u", k=4)
                dst = xb_tile.rearrange("r (k u) -> r k u", k=4)
                nc.sync.dma_start(out=dst[:, :, u0:u1], in_=src[:, :, u0:u1])

            # weights + selection matrices, chunked in consumption order
            for k in range(4):
                nc.sync.dma_start(
                    out=wk[:, k, :], in_=wt_d.rearrange("r (k p) -> r k p", k=4)[:, k, :]
                )
            nc.sync.dma_start(out=esb[:], in_=ew_d[:])

            dma_engs = [nc.sync, nc.scalar, nc.gpsimd]
            rr_state = [0]

            def level1_chunk(xb, gs, m):
                # compute chunk m's G over the full U, staged in fp16, then
                # apply the per-run q-shift with one big SBUF->SBUF DMA per
                # run (engines need 32-aligned partition bases; DMA doesn't),
                # round-robined across queues so they run in parallel
                gt = gtpool.tile([128, U], f16, tag="gt")
                for (u0, nu) in u_tiles:
                    ps = ps1pool.tile([128, 512], f32, tag="ps1")
                    for k in range(4):
                        nc.tensor.matmul(
                            ps[:, :nu],
                            lhsT=wk[:, k, m * 128:(m + 1) * 128],
                            rhs=xb[:, k * U + u0: k * U + u0 + nu],
                            start=(k == 0),
                            stop=(k == 3),
                        )
                    nc.vector.tensor_copy(gt[:, u0:u0 + nu], ps[:, :nu])
                for (r0, r1, q) in runs[m]:
                    eng = dma_engs[rr_state[0] % len(dma_engs)]
                    rr_state[0] += 1
                    eng.dma_start(
                        out=gs[r0:r1, m * T_out: (m + 1) * T_out],
                        in_=gt[r0:r1, q: q + T_out],
                    )

            def level2(gs, b, t0, nt):
                pa = ps2pool.tile([128, 512], f32, tag="pa")
                if cb:
                    pb = ps2pool.tile([128, 512], f32, tag="pb")
                for m in range(nchunks):
                    rhs = gs[:, m * T_out + t0: m * T_out + t0 + nt]
                    nc.tensor.matmul(
                        pa[:min(C, 128), :nt],
                        lhsT=esb[:, m * C: m * C + min(C, 128)],
                        rhs=rhs,
                        start=(m == 0),
                        stop=(m == nchunks - 1),
                    )
                    if cb and has_b[m]:
                        nc.tensor.matmul(
                            pb[:cb, :nt],
                            lhsT=esb[:, m * C + 128: m * C + C],
                            rhs=rhs,
                            start=(m == b_chunks[0]),
                            stop=(m == b_chunks[-1]),
                        )
                eva = evpool.tile([128, 512], f32, tag="eva")
                nc.vector.tensor_copy(eva[:min(C, 128), :nt], pa[:min(C, 128), :nt])
                nc.sync.dma_start(
                    out=out_d[b, 0:min(C, 128), t0:t0 + nt],
                    in_=eva[:min(C, 128), :nt],
                )
                if cb:
                    evb = evpool.tile([128, 512], f32, tag="evb")
                    nc.vector.tensor_copy(evb[:cb, :nt], pb[:cb, :nt])
                    nc.sync.dma_start(
                        out=out_d[b, 128:C, t0:t0 + nt], in_=evb[:cb, :nt]
                    )

            # software pipeline: emit L1(b+1) before L2(b) so the in-order
            # PE queue has work while batch b's shift DMAs drain
            gs_by_b = {}
            for b in range(b_per):
                xb = xpool.tile([128, 4 * U], f16, tag="xb", name=f"xb{b}")
                for (u0, nu) in u_tiles:
                    dma_x_chunk(xb, b, u0, u0 + nu)
                gs = gspool.tile([128, nchunks * T_out], f16, tag="gs")
                gs_by_b[b] = gs
                for m in range(nchunks):
                    level1_chunk(xb, gs, m)
                if b > 0:
                    for (t0, nt) in t_tiles:
                        level2(gs_by_b[b - 1], b - 1, t0, nt)
            for (t0, nt) in t_tiles:
                level2(gs_by_b[b_per - 1], b_per - 1, t0, nt)
    nc.compile()
    return nc


def _build_program_a(b_per, C, U, T_out, keep, Ms, offs):
    """A formulation (fallback for dense kernels): weights stationary,
    x moving, fp32r; out[b, c, t]."""
    import concourse.mybir as mybir
    import concourse.tile as tile
    from concourse import bacc

    f32 = mybir.dt.float32
    f32r = mybir.dt.float32r
    nblk = len(keep)
    sum_m = offs[-1]
    mb_max = max(max(Ms) - 128, 0)
    nts = [512] * (T_out // 512) + ([T_out % 512] if T_out % 512 else [])
    a_ps = list(range(nblk))
    b_ps = [p for p in a_ps if Ms[p] > 128]
    j_max = max(keep) // 4
    w_budgets = [192, 256, 512] + [704] * nblk
    w_chunks = []
    p0 = 0
    while p0 < nblk:
        budget = w_budgets[len(w_chunks)]
        p1 = p0 + 1
        while p1 < nblk and offs[p1 + 1] - offs[p0] <= budget:
            p1 += 1
        w_chunks.append((p0, p1))
        p0 = p1
    x_stops = []
    t0 = 0
    for nt in nts:
        x_stops.append(min(t0 + nt + j_max + 1, U))
        t0 += nt
    x_stops[-1] = U
    x_chunks = []
    u0 = 0
    for u1 in x_stops:
        if u1 > u0:
            x_chunks.append((u0, u1))
            u0 = u1

    nc = bacc.Bacc(
        "TRN2",
        target_bir_lowering=False,
        debug=False,
        enable_asserts=True,
        num_devices=N_CORES,
    )
    xt_d = nc.dram_tensor("xt", [b_per, 128, 4 * U], f32r, kind="ExternalInput").ap()
    wt_d = nc.dram_tensor("wt", [128, sum_m], f32r, kind="ExternalInput").ap()
    out_d = nc.dram_tensor("out", [b_per, C, T_out], f32, kind="ExternalOutput").ap()

    with tile.TileContext(nc) as tc:
        with (
            tc.tile_pool(name="wpool", bufs=1) as wpool,
            tc.tile_pool(name="xpool", bufs=2) as xpool,
            tc.tile_pool(name="evpool", bufs=3) as evpool,
            tc.tile_pool(name="pspool", bufs=2, space="PSUM") as pspool,
        ):
            wsb = wpool.tile([128, sum_m], f32r)

            def dma_x_chunk(xb_tile, b, u0, u1, ks):
                src = xt_d[b].rearrange("r (k u) -> r k u", k=4)
                dst = xb_tile.rearrange("r (k u) -> r k u", k=4)
                nc.sync.dma_start(
                    out=dst[:, ks[0]:ks[-1] + 1, u0:u1],
                    in_=src[:, ks[0]:ks[-1] + 1, u0:u1],
                )

            xb0 = xpool.tile([128, 4 * U], f32r, tag="xb", name="xb0")
            k_first = []
            for p in a_ps:
                k = keep[p] % 4
                if k not in k_first:
                    k_first.append(k)
            x_emits = [(x_chunks[0], (k,)) for k in k_first]
            x_emits += [(ch, (0, 1, 2, 3)) for ch in x_chunks[1:]]
            emits = []
            for i in range(max(len(x_emits), len(w_chunks))):
                if i < len(x_emits):
                    emits.append(("x", x_emits[i]))
                if i < len(w_chunks):
                    emits.append(("w", w_chunks[i]))
            for kind, args in emits:
                if kind == "x":
                    (u0, u1), ks = args
                    dma_x_chunk(xb0, 0, u0, u1, ks)
                else:
                    a0, a1 = args
                    nc.sync.dma_start(
                        out=wsb[:, offs[a0]:offs[a1]],
                        in_=wt_d[:, offs[a0]:offs[a1]],
                    )

            for b in range(b_per):
                if b == 0:
                    xb = xb0
                else:
                    xb = xpool.tile([128, 4 * U], f32r, tag="xb", name=f"xb{b}")
                    nc.sync.dma_start(out=xb[:], in_=xt_d[b])
                t0 = 0
                for nt in nts:
                    pa = pspool.tile([128, 512], f32, tag="pa")
                    if mb_max:
                        pb = pspool.tile([128, 512], f32, tag="pb")
                    for pos, p in enumerate(a_ps):
                        m = Ms[p]
                        j, k = divmod(keep[p], 4)
                        rhs = xb[:, k * U + t0 + j: k * U + t0 + j + nt]
                        ma = min(m, 128)
                        nc.tensor.matmul(
                            pa[:ma, :nt],
                            lhsT=wsb[:, offs[p]: offs[p] + ma],
                            rhs=rhs,
                            start=(pos == 0),
                            stop=(pos == len(a_ps) - 1),
                        )
                        if m > 128:
                            nc.tensor.matmul(
                                pb[:m - 128, :nt],
                                lhsT=wsb[:, offs[p] + 128: offs[p] + m],
                                rhs=rhs,
                                start=(p == b_ps[0]),
                                stop=(p == b_ps[-1]),
                            )
                    ma1 = min(Ms[a_ps[0]], 128)
                    eva = evpool.tile([128, 512], f32, tag="eva")
                    nc.vector.tensor_copy(eva[:ma1, :nt], pa[:ma1, :nt])
                    nc.sync.dma_start(
                        out=out_d[b, 0:ma1, t0:t0 + nt], in_=eva[:ma1, :nt]
                    )
                    if mb_max:
                        evb = evpool.tile([128, 512], f32, tag="evb")
                        nc.vector.tensor_copy(evb[:mb_max, :nt], pb[:mb_max, :nt])
                        nc.sync.dma_start(
                            out=out_d[b, 128:128 + mb_max, t0:t0 + nt],
                            in_=evb[:mb_max, :nt],
                        )
                    t0 += nt
    nc.compile()
    return nc


def _ensure_trace_shims():
    """If run_bass_kernel_spmd is invoked with tracing enabled (e.g. via
    BASS_TRACE=1) it imports antenv.axon_hooks and uploads artifacts to a
    bucket; neither exists in a bare container.  Register a working NTFF
    hook (ctypes into the axon .so) and a no-op uploader so the trace path
    degrades gracefully instead of crashing."""
    import sys

    try:
        import antenv.axon_hooks  # noqa: F401
    except ImportError:
        import contextlib
        import ctypes
        import types

        hook = None
        try:
            lib = ctypes.CDLL("/opt/axon/libaxon_pjrt.so")
            if hasattr(lib, "axon_start_nrt_profile"):
                lib.axon_start_nrt_profile.argtypes = [
                    ctypes.POINTER(ctypes.c_int64),
                    ctypes.c_size_t,
                ]
                lib.axon_start_nrt_profile.restype = ctypes.c_int64
                lib.axon_stop_nrt_profile.argtypes = [ctypes.c_char_p]
                lib.axon_stop_nrt_profile.restype = ctypes.c_int64

                @contextlib.contextmanager
                def _hook(output_dir, device_ids):
                    import jax

                    jax.devices()
                    if device_ids:
                        ids = (ctypes.c_int64 * len(device_ids))(*device_ids)
                        rc = lib.axon_start_nrt_profile(ids, len(device_ids))
                    else:
                        rc = lib.axon_start_nrt_profile(None, 0)
                    if rc != 0:
                        raise RuntimeError(f"axon_start_nrt_profile rc={rc}")
                    try:
                        yield
                    finally:
                        lib.axon_stop_nrt_profile(str(output_dir).encode())

                hook = _hook
        except OSError:
            pass
        mod = types.ModuleType("antenv.axon_hooks")
        mod.get_axon_ntff_profile_hook = lambda: hook
        mod.set_axon_ntff_profile_hook = lambda h: None
        sys.modules["antenv.axon_hooks"] = mod

    try:
        import concourse.bass_utils as _bu

        _orig_upload = _bu.upload_artifacts

        def _safe_upload(tmpdir):
            try:
                return _orig_upload(tmpdir)
            except Exception:
                return "local://unavailable"

        if not getattr(_bu, "_safe_upload_installed", False):
            _bu.upload_artifacts = _safe_upload
            _bu._safe_upload_installed = True
    except Exception:
        pass


def kernel(x, kernels):
    _ensure_trace_shims()
    from concourse.bass_utils import run_bass_kernel_spmd

    hp = _host_prep(x, kernels)
    xt, mode, wexp = hp["xt"], hp["mode"], hp["wexp"]
    C, U, T_out, nbins = hp["C"], hp["U"], hp["T_out"], hp["nbins"]
    B = xt.shape[0]
    assert B % N_CORES == 0
    b_per = B // N_CORES

    if mode == "D":
        nq, nchunks, runs, has_b = hp["meta"]
        runs = [list(r) for r in runs]
        key = ("D", b_per, C, U, T_out, nq, nchunks,
               tuple(tuple(t) for rr in runs for t in rr), has_b)
        if key not in _prog_cache:
            _prog_cache[key] = _build_program_d(
                b_per, C, U, T_out, nq, nchunks, runs, list(has_b)
            )
        nc = _prog_cache[key]
        in_maps = [
            {"xt": xt[c * b_per:(c + 1) * b_per], "wt": hp["wt"], "ew": hp["ew"]}
            for c in range(N_CORES)
        ]
    else:
        keep, Ms, offs = hp["keep"], hp["Ms"], hp["offs"]
        key = ("A", b_per, C, U, T_out, tuple(keep), tuple(Ms))
        if key not in _prog_cache:
            _prog_cache[key] = _build_program_a(b_per, C, U, T_out, keep, Ms, offs)
        nc = _prog_cache[key]
        in_maps = [
            {"xt": xt[c * b_per:(c + 1) * b_per], "wt": hp["wt"]}
            for c in range(N_CORES)
        ]

    res = run_bass_kernel_spmd(nc, in_maps, list(range(N_CORES)))
    parts = [res.results[c]["out"] for c in range(N_CORES)]
    out = np.concatenate(parts, axis=0)  # (B, C, T_out)
    if wexp:
        out = out * np.float32(2.0 ** -wexp)
    return np.ascontiguousarray(
        out.reshape(B, nbins, 2, T_out).transpose(0, 2, 1, 3)
    )


# revision 17
# speedup vs baseline: 2.1599x; 1.1941x over previous
"""CQT (constant-Q transform) kernel for Trainium2, 8 NeuronCores.

Math: out[b, c, t] = sum_l W[c, l] * x_pad[b, t*HOP + l]   (strided conv,
HOP=512, L=11339 taps, C=168 channels = 84 bins x re/im), then reshaped to
(B, 2, n_bins, T_out).

Strategy (two-level factorization, data-parallel over batch):
  - Write l = 512q + s.  Level 1 contracts s: with the polyphase matrix
    Y[s, u] = xp[512u + s], compute G[(c,q), u] = sum_s W[c,512q+s] Y[s,u]
    for the ~815 ACTIVE (channel, hop-block) pairs only (CQT kernels are
    ragged: bin k has ~11339*2^(-k/12) centered taps, so sum_c ceil(l_c/512)
    ~= 815 of 168*23 possible pairs).  Packed into ceil(815/128) = 7 dense
    chunks of 128 pairs -> 28 matmuls per u-sweep at N=512, ~91% PE
    utilization (vs 29% for the naive 128-tap-block decomposition).
  - Shift on evict: PSUM->SBUF copies write Gs[p, t] = G[p, t+q(p)] (pairs
    sharing q form contiguous runs, so each run is one affine copy),
    converting fp32->fp16.
  - Level 2 contracts q: out[c, t] = sum_p E[p, c] Gs[p, t] with one-hot
    E per chunk -- 7+1 matmuls of N=512 per t-tile into the usual
    [C, t] PSUM layout.
  - fp16 x and 2^wexp-scaled fp16 W (undone on host); fp32 PSUM.
  - A formulation (weights stationary, x moving, fp32r) kept as fallback
    if the kernel tensor has no zero raggedness.
"""

import numpy as np

HOP = 512
N_CORES = 8

_prog_cache: dict = {}


def _host_prep(x, kernels):
    x = np.ascontiguousarray(np.asarray(x, dtype=np.float32))
    kernels = np.ascontiguousarray(np.asarray(kernels, dtype=np.float32))
    B, T = x.shape
    nbins, two, Lmax = kernels.shape
    assert two == 2
    C = 2 * nbins
    pad = Lmax // 2
    T_out = (T + 2 * pad - Lmax) // HOP + 1

    nblk_full = -(-Lmax // 128)
    nq = -(-(nblk_full * 128) // 512)
    Wp = np.zeros((C, nq * 512), dtype=np.float32)
    Wp[:, :Lmax] = kernels.reshape(C, Lmax)

    # ---- active (q, c) pairs for the D formulation ----
    seg_nz = (Wp.reshape(C, nq, 512) != 0.0).any(axis=2)  # [C, nq]
    # cluster c>=128 pairs at the end so they occupy the fewest chunks
    # (each chunk holding such pairs costs an extra matmul per t-tile)
    pairs = sorted(
        ((q, c) for q in range(nq) for c in range(C) if seg_nz[c, q]),
        key=lambda qc: (qc[1] >= 128, qc[0], qc[1]),
    )
    npairs = len(pairs)
    nchunks = -(-npairs // 128)
    npad = nchunks * 128

    # ---- cost model: D (two-level) vs A (tap-block, W-stationary) ----
    nzb = (Wp[:, :nblk_full * 128].reshape(C, nblk_full, 128) != 0.0).any(axis=2)
    Msb, keepb = [], []
    for i in range(nblk_full):
        idx = np.where(nzb[:, i])[0]
        if len(idx):
            keepb.append(i)
            Msb.append(int(idx[-1]) + 1)
    j_max_a = (max(keepb) // 4) if keepb else 0
    U = T_out + max(nq - 1, j_max_a)
    cost_d = nchunks * 4 * U + (nchunks + (1 if C > 128 else 0)) * T_out
    cost_a = (len(keepb) + sum(1 for m in Msb if m > 128)) * T_out
    mode = "D" if (cost_d < cost_a and C <= 256) else "A"

    if mode == "D":
        U = T_out + nq - 1
        wmax = float(np.abs(Wp).max())
        wexp = int(np.floor(np.log2(0.25 / wmax))) if wmax > 0 else 0
        scale = np.float32(2.0 ** wexp)
        # wq[r, k, p] = Wp[c, 512q + 128k + r] * scale
        wq = np.zeros((128, 4, npad), dtype=np.float16)
        ee = np.zeros((128, nchunks * C), dtype=np.float16)
        runs = [[] for _ in range(nchunks)]
        has_b = [False] * nchunks
        for p, (q, c) in enumerate(pairs + [(0, 0)] * (npad - npairs)):
            m, r = divmod(p, 128)
            if p < npairs:
                wq[:, :, p] = (Wp[c, 512 * q: 512 * (q + 1)] * scale).reshape(4, 128).T
                ee[r, m * C + c] = 2.0 ** -wexp  # folds the W-scale undo into L2
                if c >= 128:
                    has_b[m] = True
            if runs[m] and runs[m][-1][2] == q:
                runs[m][-1] = (runs[m][-1][0], r + 1, q)
            else:
                runs[m].append((r, r + 1, q))
        wq = np.ascontiguousarray(wq.reshape(128, 4 * npad))
        wexp = 0  # already undone via ee
        xdt = np.float16
        meta = (nq, nchunks, tuple(
            tuple(rr) for rr in ((tuple(t) for t in runs[m]) for m in range(nchunks))
        ), tuple(has_b))
        wt, keep, Ms, offs = wq, None, None, None
        ew = ee
    else:
        wexp = 0
        # ragged 128-tap blocks, desc active-prefix order (A path)
        keep = np.asarray(keepb, dtype=np.int64)
        Ms = np.asarray(Msb, dtype=np.int64)
        order = np.argsort(-Ms, kind="stable")
        keep = keep[order]
        Ms = Ms[order]
        wblk = Wp[:, :nblk_full * 128].reshape(C, nblk_full, 128)
        wt = np.ascontiguousarray(
            np.concatenate([wblk[:m, i, :].T for i, m in zip(keep, Ms)], axis=1)
        )
        offs = np.concatenate([[0], np.cumsum(Ms)]).tolist()
        keep = keep.tolist()
        Ms = Ms.tolist()
        U = T_out + max(keep) // 4
        xdt = np.float32
        meta = None
        ew = None

    xpad_len = 512 * U
    assert xpad_len >= pad + T, (xpad_len, pad + T)
    xp = np.zeros((B, xpad_len), dtype=xdt)
    xp[:, pad:pad + T] = x.astype(xdt)
    # xt[b, r, k*U + u] = xp[b, 512u + 128k + r]
    xt = np.ascontiguousarray(
        xp.reshape(B, U, 4, 128).transpose(0, 3, 2, 1).reshape(B, 128, 4 * U)
    )
    return dict(
        xt=xt, wt=wt, ew=ew, keep=keep, Ms=Ms, offs=offs, C=C, U=U,
        T_out=T_out, nbins=nbins, mode=mode, wexp=wexp, meta=meta,
    )


def _tiles(total, step):
    return [(t0, min(step, total - t0)) for t0 in range(0, total, step)]


def _build_program_d(b_per, C, U, T_out, nq, nchunks, runs, has_b):
    import concourse.mybir as mybir
    import concourse.tile as tile
    from concourse import bacc

    f32 = mybir.dt.float32
    f16 = mybir.dt.float16
    npad = nchunks * 128
    cb = C - 128 if C > 128 else 0
    u_tiles = _tiles(U, 512)
    t_tiles = _tiles(T_out, 512)
    b_chunks = [m for m in range(nchunks) if has_b[m]]

    nc = bacc.Bacc(
        "TRN2",
        target_bir_lowering=False,
        debug=False,
        enable_asserts=True,
        num_devices=N_CORES,
    )
    xt_d = nc.dram_tensor("xt", [b_per, 128, 4 * U], f16, kind="ExternalInput").ap()
    wt_d = nc.dram_tensor("wt", [128, 4 * npad], f16, kind="ExternalInput").ap()
    ew_d = nc.dram_tensor("ew", [128, nchunks * C], f16, kind="ExternalInput").ap()
    out_d = nc.dram_tensor("out", [b_per, C, T_out], f32, kind="ExternalOutput").ap()

    with tile.TileContext(nc) as tc:
        with (
            tc.tile_pool(name="wpool", bufs=1) as wpool,
            tc.tile_pool(name="xpool", bufs=2) as xpool,
            tc.tile_pool(name="gspool", bufs=2) as gspool,
            tc.tile_pool(name="gtpool", bufs=4) as gtpool,
            tc.tile_pool(name="evpool", bufs=3) as evpool,
            tc.tile_pool(name="ps1pool", bufs=4, space="PSUM") as ps1pool,
            tc.tile_pool(name="ps2pool", bufs=2, space="PSUM") as ps2pool,
        ):
            wsb = wpool.tile([128, 4 * npad], f16)
            esb = wpool.tile([128, nchunks * C], f16)
            wk = wsb.rearrange("r (k p) -> r k p", k=4)

            def dma_x_chunk(xb_tile, b, u0, u1, eng):
                src = xt_d[b].rearrange("r (k u) -> r k u", k=4)
                dst = xb_tile.rearrange("r (k u) -> r k u", k=4)
                eng.dma_start(out=dst[:, :, u0:u1], in_=src[:, :, u0:u1])

            # weights + selection matrices, spread across DMA queues so the
            # first chunk's dependencies land concurrently
            for k in range(4):
                nc.scalar.dma_start(
                    out=wk[:, k, :], in_=wt_d.rearrange("r (k p) -> r k p", k=4)[:, k, :]
                )
            nc.gpsimd.dma_start(out=esb[:], in_=ew_d[:])

            dma_engs = [nc.sync, nc.scalar, nc.gpsimd]
            rr_state = [0]

            def level1_chunk(xb, gs, m):
                # compute chunk m's G over the full U, staged in fp16, then
                # apply the per-run q-shift with one big SBUF->SBUF DMA per
                # run (engines need 32-aligned partition bases; DMA doesn't),
                # round-robined across queues so they run in parallel
                gt = gtpool.tile([128, U], f16, tag="gt")
                for (u0, nu) in u_tiles:
                    ps = ps1pool.tile([128, 512], f32, tag="ps1")
                    for k in range(4):
                        nc.tensor.matmul(
                            ps[:, :nu],
                            lhsT=wk[:, k, m * 128:(m + 1) * 128],
                            rhs=xb[:, k * U + u0: k * U + u0 + nu],
                            start=(k == 0),
                            stop=(k == 3),
                        )
                    nc.vector.tensor_copy(gt[:, u0:u0 + nu], ps[:, :nu])
                for (r0, r1, q) in runs[m]:
                    eng = dma_engs[rr_state[0] % len(dma_engs)]
                    rr_state[0] += 1
                    eng.dma_start(
                        out=gs[r0:r1, m * T_out: (m + 1) * T_out],
                        in_=gt[r0:r1, q: q + T_out],
                    )

            def level2(gs, b, t0, nt):
                pa = ps2pool.tile([128, 512], f32, tag="pa")
                if cb:
                    pb = ps2pool.tile([128, 512], f32, tag="pb")
                for m in range(nchunks):
                    rhs = gs[:, m * T_out + t0: m * T_out + t0 + nt]
                    nc.tensor.matmul(
                        pa[:min(C, 128), :nt],
                        lhsT=esb[:, m * C: m * C + min(C, 128)],
                        rhs=rhs,
                        start=(m == 0),
                        stop=(m == nchunks - 1),
                    )
                    if cb and has_b[m]:
                        nc.tensor.matmul(
                            pb[:cb, :nt],
                            lhsT=esb[:, m * C + 128: m * C + C],
                            rhs=rhs,
                            start=(m == b_chunks[0]),
                            stop=(m == b_chunks[-1]),
                        )
                eva = evpool.tile([128, 512], f32, tag="eva")
                nc.vector.tensor_copy(eva[:min(C, 128), :nt], pa[:min(C, 128), :nt])
                nc.sync.dma_start(
                    out=out_d[b, 0:min(C, 128), t0:t0 + nt],
                    in_=eva[:min(C, 128), :nt],
                )
                if cb:
                    evb = evpool.tile([128, 512], f32, tag="evb")
                    nc.vector.tensor_copy(evb[:cb, :nt], pb[:cb, :nt])
                    nc.sync.dma_start(
                        out=out_d[b, 128:C, t0:t0 + nt], in_=evb[:cb, :nt]
                    )

            # software pipeline: emit L1(b+1) before L2(b) so the in-order
            # PE queue has work while batch b's shift DMAs drain
            gs_by_b = {}
            for b in range(b_per):
                xb = xpool.tile([128, 4 * U], f16, tag="xb", name=f"xb{b}")
                for ui, (u0, nu) in enumerate(u_tiles):
                    dma_x_chunk(xb, b, u0, u0 + nu, dma_engs[ui % len(dma_engs)])
                gs = gspool.tile([128, nchunks * T_out], f16, tag="gs")
                gs_by_b[b] = gs
                for m in range(nchunks):
                    level1_chunk(xb, gs, m)
                if b > 0:
                    for (t0, nt) in t_tiles:
                        level2(gs_by_b[b - 1], b - 1, t0, nt)
            for (t0, nt) in t_tiles:
                level2(gs_by_b[b_per - 1], b_per - 1, t0, nt)
    nc.compile()
    return nc


def _build_program_a(b_per, C, U, T_out, keep, Ms, offs):
    """A formulation (fallback for dense kernels): weights stationary,
    x moving, fp32r; out[b, c, t]."""
    import concourse.mybir as mybir
    import concourse.tile as tile
    from concourse import bacc

    f32 = mybir.dt.float32
    f32r = mybir.dt.float32r
    nblk = len(keep)
    sum_m = offs[-1]
    mb_max = max(max(Ms) - 128, 0)
    nts = [512] * (T_out // 512) + ([T_out % 512] if T_out % 512 else [])
    a_ps = list(range(nblk))
    b_ps = [p for p in a_ps if Ms[p] > 128]
    j_max = max(keep) // 4
    w_budgets = [192, 256, 512] + [704] * nblk
    w_chunks = []
    p0 = 0
    while p0 < nblk:
        budget = w_budgets[len(w_chunks)]
        p1 = p0 + 1
        while p1 < nblk and offs[p1 + 1] - offs[p0] <= budget:
            p1 += 1
        w_chunks.append((p0, p1))
        p0 = p1
    x_stops = []
    t0 = 0
    for nt in nts:
        x_stops.append(min(t0 + nt + j_max + 1, U))
        t0 += nt
    x_stops[-1] = U
    x_chunks = []
    u0 = 0
    for u1 in x_stops:
        if u1 > u0:
            x_chunks.append((u0, u1))
            u0 = u1

    nc = bacc.Bacc(
        "TRN2",
        target_bir_lowering=False,
        debug=False,
        enable_asserts=True,
        num_devices=N_CORES,
    )
    xt_d = nc.dram_tensor("xt", [b_per, 128, 4 * U], f32r, kind="ExternalInput").ap()
    wt_d = nc.dram_tensor("wt", [128, sum_m], f32r, kind="ExternalInput").ap()
    out_d = nc.dram_tensor("out", [b_per, C, T_out], f32, kind="ExternalOutput").ap()

    with tile.TileContext(nc) as tc:
        with (
            tc.tile_pool(name="wpool", bufs=1) as wpool,
            tc.tile_pool(name="xpool", bufs=2) as xpool,
            tc.tile_pool(name="evpool", bufs=3) as evpool,
            tc.tile_pool(name="pspool", bufs=2, space="PSUM") as pspool,
        ):
            wsb = wpool.tile([128, sum_m], f32r)

            def dma_x_chunk(xb_tile, b, u0, u1, ks):
                src = xt_d[b].rearrange("r (k u) -> r k u", k=4)
                dst = xb_tile.rearrange("r (k u) -> r k u", k=4)
                nc.sync.dma_start(
                    out=dst[:, ks[0]:ks[-1] + 1, u0:u1],
                    in_=src[:, ks[0]:ks[-1] + 1, u0:u1],
                )

            xb0 = xpool.tile([128, 4 * U], f32r, tag="xb", name="xb0")
            k_first = []
            for p in a_ps:
                k = keep[p] % 4
                if k not in k_first:
                    k_first.append(k)
            x_emits = [(x_chunks[0], (k,)) for k in k_first]
            x_emits += [(ch, (0, 1, 2, 3)) for ch in x_chunks[1:]]
            emits = []
            for i in range(max(len(x_emits), len(w_chunks))):
                if i < len(x_emits):
                    emits.append(("x", x_emits[i]))
                if i < len(w_chunks):
                    emits.append(("w", w_chunks[i]))
            for kind, args in emits:
                if kind == "x":
                    (u0, u1), ks = args
                    dma_x_chunk(xb0, 0, u0, u1, ks)
                else:
                    a0, a1 = args
                    nc.sync.dma_start(
                        out=wsb[:, offs[a0]:offs[a1]],
                        in_=wt_d[:, offs[a0]:offs[a1]],
                    )

            for b in range(b_per):
                if b == 0:
                    xb = xb0
                else:
                    xb = xpool.tile([128, 4 * U], f32r, tag="xb", name=f"xb{b}")
                    nc.sync.dma_start(out=xb[:], in_=xt_d[b])
                t0 = 0
                for nt in nts:
                    pa = pspool.tile([128, 512], f32, tag="pa")
                    if mb_max:
                        pb = pspool.tile([128, 512], f32, tag="pb")
                    for pos, p in enumerate(a_ps):
                        m = Ms[p]
                        j, k = divmod(keep[p], 4)
                        rhs = xb[:, k * U + t0 + j: k * U + t0 + j + nt]
                        ma = min(m, 128)
                        nc.tensor.matmul(
                            pa[:ma, :nt],
                            lhsT=wsb[:, offs[p]: offs[p] + ma],
                            rhs=rhs,
                            start=(pos == 0),
                            stop=(pos == len(a_ps) - 1),
                        )
                        if m > 128:
                            nc.tensor.matmul(
                                pb[:m - 128, :nt],
                                lhsT=wsb[:, offs[p] + 128: offs[p] + m],
                                rhs=rhs,
                                start=(p == b_ps[0]),
                                stop=(p == b_ps[-1]),
                            )
                    ma1 = min(Ms[a_ps[0]], 128)
                    eva = evpool.tile([128, 512], f32, tag="eva")
                    nc.vector.tensor_copy(eva[:ma1, :nt], pa[:ma1, :nt])
                    nc.sync.dma_start(
                        out=out_d[b, 0:ma1, t0:t0 + nt], in_=eva[:ma1, :nt]
                    )
                    if mb_max:
                        evb = evpool.tile([128, 512], f32, tag="evb")
                        nc.vector.tensor_copy(evb[:mb_max, :nt], pb[:mb_max, :nt])
                        nc.sync.dma_start(
                            out=out_d[b, 128:128 + mb_max, t0:t0 + nt],
                            in_=evb[:mb_max, :nt],
                        )
                    t0 += nt
    nc.compile()
    return nc


def _ensure_trace_shims():
    """If run_bass_kernel_spmd is invoked with tracing enabled (e.g. via
    BASS_TRACE=1) it imports antenv.axon_hooks and uploads artifacts to a
    bucket; neither exists in a bare container.  Register a working NTFF
    hook (ctypes into the axon .so) and a no-op uploader so the trace path
    degrades gracefully instead of crashing."""
    import sys

    try:
        import antenv.axon_hooks  # noqa: F401
    except ImportError:
        import contextlib
        import ctypes
        import types

        hook = None
        try:
            lib = ctypes.CDLL("/opt/axon/libaxon_pjrt.so")
            if hasattr(lib, "axon_start_nrt_profile"):
                lib.axon_start_nrt_profile.argtypes = [
                    ctypes.POINTER(ctypes.c_int64),
                    ctypes.c_size_t,
                ]
                lib.axon_start_nrt_profile.restype = ctypes.c_int64
                lib.axon_stop_nrt_profile.argtypes = [ctypes.c_char_p]
                lib.axon_stop_nrt_profile.restype = ctypes.c_int64

                @contextlib.contextmanager
                def _hook(output_dir, device_ids):
                    import jax

                    jax.devices()
                    if device_ids:
                        ids = (ctypes.c_int64 * len(device_ids))(*device_ids)
                        rc = lib.axon_start_nrt_profile(ids, len(device_ids))
                    else:
                        rc = lib.axon_start_nrt_profile(None, 0)
                    if rc != 0:
                        raise RuntimeError(f"axon_start_nrt_profile rc={rc}")
                    try:
                        yield
                    finally:
                        lib.axon_stop_nrt_profile(str(output_dir).encode())

                hook = _hook
        except OSError:
            pass
        mod = types.ModuleType("antenv.axon_hooks")
        mod.get_axon_ntff_profile_hook = lambda: hook
        mod.set_axon_ntff_profile_hook = lambda h: None
        sys.modules["antenv.axon_hooks"] = mod

    try:
        import concourse.bass_utils as _bu

        _orig_upload = _bu.upload_artifacts

        def _safe_upload(tmpdir):
            try:
                return _orig_upload(tmpdir)
            except Exception:
                return "local://unavailable"

        if not getattr(_bu, "_safe_upload_installed", False):
            _bu.upload_artifacts = _safe_upload
            _bu._safe_upload_installed = True
    except Exception:
        pass


def kernel(x, kernels):
    _ensure_trace_shims()
    from concourse.bass_utils import run_bass_kernel_spmd

    hp = _host_prep(x, kernels)
    xt, mode, wexp = hp["xt"], hp["mode"], hp["wexp"]
    C, U, T_out, nbins = hp["C"], hp["U"], hp["T_out"], hp["nbins"]
    B = xt.shape[0]
    assert B % N_CORES == 0
    b_per = B // N_CORES

    if mode == "D":
        nq, nchunks, runs, has_b = hp["meta"]
        runs = [list(r) for r in runs]
        key = ("D", b_per, C, U, T_out, nq, nchunks,
               tuple(tuple(t) for rr in runs for t in rr), has_b)
        if key not in _prog_cache:
            _prog_cache[key] = _build_program_d(
                b_per, C, U, T_out, nq, nchunks, runs, list(has_b)
            )
        nc = _prog_cache[key]
        in_maps = [
            {"xt": xt[c * b_per:(c + 1) * b_per], "wt": hp["wt"], "ew": hp["ew"]}
            for c in range(N_CORES)
        ]
    else:
        keep, Ms, offs = hp["keep"], hp["Ms"], hp["offs"]
        key = ("A", b_per, C, U, T_out, tuple(keep), tuple(Ms))
        if key not in _prog_cache:
            _prog_cache[key] = _build_program_a(b_per, C, U, T_out, keep, Ms, offs)
        nc = _prog_cache[key]
        in_maps = [
            {"xt": xt[c * b_per:(c + 1) * b_per], "wt": hp["wt"]}
            for c in range(N_CORES)
        ]

    res = run_bass_kernel_spmd(nc, in_maps, list(range(N_CORES)))
    parts = [res.results[c]["out"] for c in range(N_CORES)]
    out = np.concatenate(parts, axis=0)  # (B, C, T_out)
    if wexp:
        out = out * np.float32(2.0 ** -wexp)
    return np.ascontiguousarray(
        out.reshape(B, nbins, 2, T_out).transpose(0, 2, 1, 3)
    )


# revision 20
# speedup vs baseline: 2.3833x; 1.1035x over previous
"""CQT (constant-Q transform) kernel for Trainium2, 8 NeuronCores.

Math: out[b, c, t] = sum_l W[c, l] * x_pad[b, t*HOP + l]   (strided conv,
HOP=512, L=11339 taps, C=168 channels = 84 bins x re/im), then reshaped to
(B, 2, n_bins, T_out).

Strategy (two-level factorization, data-parallel over batch):
  - Write l = 512q + s.  Level 1 contracts s: with the polyphase matrix
    Y[s, u] = xp[512u + s], compute G[(c,q), u] = sum_s W[c,512q+s] Y[s,u]
    for the ~815 ACTIVE (channel, hop-block) pairs only (CQT kernels are
    ragged: bin k has ~11339*2^(-k/12) centered taps, so sum_c ceil(l_c/512)
    ~= 815 of 168*23 possible pairs).  Packed into ceil(815/128) = 7 dense
    chunks of 128 pairs -> 28 matmuls per u-sweep at N=512, ~91% PE
    utilization (vs 29% for the naive 128-tap-block decomposition).
  - Shift on evict: PSUM->SBUF copies write Gs[p, t] = G[p, t+q(p)] (pairs
    sharing q form contiguous runs, so each run is one affine copy),
    converting fp32->fp16.
  - Level 2 contracts q: out[c, t] = sum_p E[p, c] Gs[p, t] with one-hot
    E per chunk -- 7+1 matmuls of N=512 per t-tile into the usual
    [C, t] PSUM layout.
  - fp16 x and 2^wexp-scaled fp16 W (undone on host); fp32 PSUM.
  - A formulation (weights stationary, x moving, fp32r) kept as fallback
    if the kernel tensor has no zero raggedness.
"""

import numpy as np

HOP = 512
N_CORES = 8

_prog_cache: dict = {}


def _host_prep(x, kernels):
    x = np.ascontiguousarray(np.asarray(x, dtype=np.float32))
    kernels = np.ascontiguousarray(np.asarray(kernels, dtype=np.float32))
    B, T = x.shape
    nbins, two, Lmax = kernels.shape
    assert two == 2
    C = 2 * nbins
    pad = Lmax // 2
    T_out = (T + 2 * pad - Lmax) // HOP + 1

    nblk_full = -(-Lmax // 128)
    nq = -(-(nblk_full * 128) // 512)
    Wp = np.zeros((C, nq * 512), dtype=np.float32)
    Wp[:, :Lmax] = kernels.reshape(C, Lmax)

    # ---- active (q, c) pairs for the D formulation ----
    seg_nz = (Wp.reshape(C, nq, 512) != 0.0).any(axis=2)  # [C, nq]
    # cluster c>=128 pairs at the end so they occupy the fewest chunks
    # (each chunk holding such pairs costs an extra matmul per t-tile)
    pairs = sorted(
        ((q, c) for q in range(nq) for c in range(C) if seg_nz[c, q]),
        key=lambda qc: (qc[1] >= 128, qc[0], qc[1]),
    )
    npairs = len(pairs)
    nchunks = -(-npairs // 128)
    npad = nchunks * 128

    # ---- cost model: D (two-level) vs A (tap-block, W-stationary) ----
    nzb = (Wp[:, :nblk_full * 128].reshape(C, nblk_full, 128) != 0.0).any(axis=2)
    Msb, keepb = [], []
    for i in range(nblk_full):
        idx = np.where(nzb[:, i])[0]
        if len(idx):
            keepb.append(i)
            Msb.append(int(idx[-1]) + 1)
    j_max_a = (max(keepb) // 4) if keepb else 0
    U = T_out + max(nq - 1, j_max_a)
    cost_d = nchunks * 4 * U + (nchunks + (1 if C > 128 else 0)) * T_out
    cost_a = (len(keepb) + sum(1 for m in Msb if m > 128)) * T_out
    mode = "D" if (cost_d < cost_a and C <= 256) else "A"

    if mode == "D":
        U = T_out + nq - 1
        wmax = float(np.abs(Wp).max())
        wexp = int(np.floor(np.log2(0.25 / wmax))) if wmax > 0 else 0
        scale = np.float32(2.0 ** wexp)
        # wq[r, k, p] = Wp[c, 512q + 128k + r] * scale
        wq = np.zeros((128, 4, npad), dtype=np.float16)
        ee = np.zeros((128, nchunks * C), dtype=np.float16)
        runs = [[] for _ in range(nchunks)]
        has_b = [False] * nchunks
        for p, (q, c) in enumerate(pairs + [(0, 0)] * (npad - npairs)):
            m, r = divmod(p, 128)
            if p < npairs:
                wq[:, :, p] = (Wp[c, 512 * q: 512 * (q + 1)] * scale).reshape(4, 128).T
                ee[r, m * C + c] = 2.0 ** -wexp  # folds the W-scale undo into L2
                if c >= 128:
                    has_b[m] = True
            if runs[m] and runs[m][-1][2] == q:
                runs[m][-1] = (runs[m][-1][0], r + 1, q)
            else:
                runs[m].append((r, r + 1, q))
        wq = np.ascontiguousarray(wq.reshape(128, 4 * npad))
        wexp = 0  # already undone via ee
        xdt = np.float16
        meta = (nq, nchunks, tuple(
            tuple(rr) for rr in ((tuple(t) for t in runs[m]) for m in range(nchunks))
        ), tuple(has_b))
        wt, keep, Ms, offs = wq, None, None, None
        ew = ee
    else:
        wexp = 0
        # ragged 128-tap blocks, desc active-prefix order (A path)
        keep = np.asarray(keepb, dtype=np.int64)
        Ms = np.asarray(Msb, dtype=np.int64)
        order = np.argsort(-Ms, kind="stable")
        keep = keep[order]
        Ms = Ms[order]
        wblk = Wp[:, :nblk_full * 128].reshape(C, nblk_full, 128)
        wt = np.ascontiguousarray(
            np.concatenate([wblk[:m, i, :].T for i, m in zip(keep, Ms)], axis=1)
        )
        offs = np.concatenate([[0], np.cumsum(Ms)]).tolist()
        keep = keep.tolist()
        Ms = Ms.tolist()
        U = T_out + max(keep) // 4
        xdt = np.float32
        meta = None
        ew = None

    xpad_len = 512 * U
    assert xpad_len >= pad + T, (xpad_len, pad + T)
    xp = np.zeros((B, xpad_len), dtype=xdt)
    xp[:, pad:pad + T] = x.astype(xdt)
    # xt[b, r, k*U + u] = xp[b, 512u + 128k + r]
    xt = np.ascontiguousarray(
        xp.reshape(B, U, 4, 128).transpose(0, 3, 2, 1).reshape(B, 128, 4 * U)
    )
    return dict(
        xt=xt, wt=wt, ew=ew, keep=keep, Ms=Ms, offs=offs, C=C, U=U,
        T_out=T_out, nbins=nbins, mode=mode, wexp=wexp, meta=meta,
    )


def _tiles(total, step):
    return [(t0, min(step, total - t0)) for t0 in range(0, total, step)]


def _build_program_d(b_per, C, U, T_out, nq, nchunks, runs, has_b):
    import concourse.mybir as mybir
    import concourse.tile as tile
    from concourse import bacc

    f32 = mybir.dt.float32
    f16 = mybir.dt.float16
    npad = nchunks * 128
    cb = C - 128 if C > 128 else 0
    u_tiles = _tiles(U, 512)
    t_tiles = _tiles(T_out, 512)
    b_chunks = [m for m in range(nchunks) if has_b[m]]

    nc = bacc.Bacc(
        "TRN2",
        target_bir_lowering=False,
        debug=False,
        enable_asserts=True,
        num_devices=N_CORES,
    )
    xt_d = nc.dram_tensor("xt", [b_per, 128, 4 * U], f16, kind="ExternalInput").ap()
    wt_d = nc.dram_tensor("wt", [128, 4 * npad], f16, kind="ExternalInput").ap()
    ew_d = nc.dram_tensor("ew", [128, nchunks * C], f16, kind="ExternalInput").ap()
    out_d = nc.dram_tensor("out", [b_per, C, T_out], f32, kind="ExternalOutput").ap()

    with tile.TileContext(nc) as tc:
        with (
            tc.tile_pool(name="wpool", bufs=1) as wpool,
            tc.tile_pool(name="xpool", bufs=2) as xpool,
            tc.tile_pool(name="gspool", bufs=2) as gspool,
            tc.tile_pool(name="gtpool", bufs=4) as gtpool,
            tc.tile_pool(name="evpool", bufs=3) as evpool,
            tc.tile_pool(name="ps1pool", bufs=4, space="PSUM") as ps1pool,
            tc.tile_pool(name="ps2pool", bufs=2, space="PSUM") as ps2pool,
        ):
            wsb = wpool.tile([128, 4 * npad], f16)
            esb = wpool.tile([128, nchunks * C], f16)
            wk = wsb.rearrange("r (k p) -> r k p", k=4)

            def dma_x_chunk(xb_tile, b, u0, u1, eng):
                src = xt_d[b].rearrange("r (k u) -> r k u", k=4)
                dst = xb_tile.rearrange("r (k u) -> r k u", k=4)
                eng.dma_start(out=dst[:, :, u0:u1], in_=src[:, :, u0:u1])

            # weights + selection matrices (scalar/gpsimd queues are still
            # empty at program start, so spread the initial loads)
            for k in range(4):
                nc.scalar.dma_start(
                    out=wk[:, k, :], in_=wt_d.rearrange("r (k p) -> r k p", k=4)[:, k, :]
                )
            nc.gpsimd.dma_start(out=esb[:], in_=ew_d[:])

            # dedicated queues: sync carries I/O (x in, out back); scalar +
            # gpsimd carry the Gs shift copies.  Queues are FIFO, so mixing
            # dependency-stalled shift DMAs with I/O causes head-of-line
            # blocking.
            dma_engs = [nc.scalar, nc.gpsimd]
            rr_state = [0]

            def level1_chunk(xb, gs, m):
                # compute chunk m's G over the full U, staged in fp16, then
                # apply the per-run q-shift with one big SBUF->SBUF DMA per
                # run (engines need 32-aligned partition bases; DMA doesn't),
                # round-robined across queues so they run in parallel
                gt = gtpool.tile([128, U], f16, tag="gt")
                for (u0, nu) in u_tiles:
                    ps = ps1pool.tile([128, 512], f32, tag="ps1")
                    for k in range(4):
                        nc.tensor.matmul(
                            ps[:, :nu],
                            lhsT=wk[:, k, m * 128:(m + 1) * 128],
                            rhs=xb[:, k * U + u0: k * U + u0 + nu],
                            start=(k == 0),
                            stop=(k == 3),
                        )
                    nc.vector.tensor_copy(gt[:, u0:u0 + nu], ps[:, :nu])
                for (r0, r1, q) in runs[m]:
                    eng = dma_engs[rr_state[0] % len(dma_engs)]
                    rr_state[0] += 1
                    eng.dma_start(
                        out=gs[r0:r1, m * T_out: (m + 1) * T_out],
                        in_=gt[r0:r1, q: q + T_out],
                    )

            def level2(gs, b, t0, nt):
                pa = ps2pool.tile([128, 512], f32, tag="pa")
                if cb:
                    pb = ps2pool.tile([128, 512], f32, tag="pb")
                for m in range(nchunks):
                    rhs = gs[:, m * T_out + t0: m * T_out + t0 + nt]
                    nc.tensor.matmul(
                        pa[:min(C, 128), :nt],
                        lhsT=esb[:, m * C: m * C + min(C, 128)],
                        rhs=rhs,
                        start=(m == 0),
                        stop=(m == nchunks - 1),
                    )
                    if cb and has_b[m]:
                        nc.tensor.matmul(
                            pb[:cb, :nt],
                            lhsT=esb[:, m * C + 128: m * C + C],
                            rhs=rhs,
                            start=(m == b_chunks[0]),
                            stop=(m == b_chunks[-1]),
                        )
                eva = evpool.tile([128, 512], f32, tag="eva")
                nc.vector.tensor_copy(eva[:min(C, 128), :nt], pa[:min(C, 128), :nt])
                nc.sync.dma_start(
                    out=out_d[b, 0:min(C, 128), t0:t0 + nt],
                    in_=eva[:min(C, 128), :nt],
                )
                if cb:
                    evb = evpool.tile([128, 512], f32, tag="evb")
                    nc.vector.tensor_copy(evb[:cb, :nt], pb[:cb, :nt])
                    nc.sync.dma_start(
                        out=out_d[b, 128:C, t0:t0 + nt], in_=evb[:cb, :nt]
                    )

            # software pipeline: emit L1(b+1) before L2(b) so the in-order
            # PE queue has work while batch b's shift DMAs drain
            gs_by_b = {}
            for b in range(b_per):
                xb = xpool.tile([128, 4 * U], f16, tag="xb", name=f"xb{b}")
                for (u0, nu) in u_tiles:
                    dma_x_chunk(xb, b, u0, u0 + nu, nc.sync)
                gs = gspool.tile([128, nchunks * T_out], f16, tag="gs")
                gs_by_b[b] = gs
                for m in range(nchunks):
                    level1_chunk(xb, gs, m)
                if b > 0:
                    for (t0, nt) in t_tiles:
                        level2(gs_by_b[b - 1], b - 1, t0, nt)
            for (t0, nt) in t_tiles:
                level2(gs_by_b[b_per - 1], b_per - 1, t0, nt)
    nc.compile()
    return nc


def _build_program_a(b_per, C, U, T_out, keep, Ms, offs):
    """A formulation (fallback for dense kernels): weights stationary,
    x moving, fp32r; out[b, c, t]."""
    import concourse.mybir as mybir
    import concourse.tile as tile
    from concourse import bacc

    f32 = mybir.dt.float32
    f32r = mybir.dt.float32r
    nblk = len(keep)
    sum_m = offs[-1]
    mb_max = max(max(Ms) - 128, 0)
    nts = [512] * (T_out // 512) + ([T_out % 512] if T_out % 512 else [])
    a_ps = list(range(nblk))
    b_ps = [p for p in a_ps if Ms[p] > 128]
    j_max = max(keep) // 4
    w_budgets = [192, 256, 512] + [704] * nblk
    w_chunks = []
    p0 = 0
    while p0 < nblk:
        budget = w_budgets[len(w_chunks)]
        p1 = p0 + 1
        while p1 < nblk and offs[p1 + 1] - offs[p0] <= budget:
            p1 += 1
        w_chunks.append((p0, p1))
        p0 = p1
    x_stops = []
    t0 = 0
    for nt in nts:
        x_stops.append(min(t0 + nt + j_max + 1, U))
        t0 += nt
    x_stops[-1] = U
    x_chunks = []
    u0 = 0
    for u1 in x_stops:
        if u1 > u0:
            x_chunks.append((u0, u1))
            u0 = u1

    nc = bacc.Bacc(
        "TRN2",
        target_bir_lowering=False,
        debug=False,
        enable_asserts=True,
        num_devices=N_CORES,
    )
    xt_d = nc.dram_tensor("xt", [b_per, 128, 4 * U], f32r, kind="ExternalInput").ap()
    wt_d = nc.dram_tensor("wt", [128, sum_m], f32r, kind="ExternalInput").ap()
    out_d = nc.dram_tensor("out", [b_per, C, T_out], f32, kind="ExternalOutput").ap()

    with tile.TileContext(nc) as tc:
        with (
            tc.tile_pool(name="wpool", bufs=1) as wpool,
            tc.tile_pool(name="xpool", bufs=2) as xpool,
            tc.tile_pool(name="evpool", bufs=3) as evpool,
            tc.tile_pool(name="pspool", bufs=2, space="PSUM") as pspool,
        ):
            wsb = wpool.tile([128, sum_m], f32r)

            def dma_x_chunk(xb_tile, b, u0, u1, ks):
                src = xt_d[b].rearrange("r (k u) -> r k u", k=4)
                dst = xb_tile.rearrange("r (k u) -> r k u", k=4)
                nc.sync.dma_start(
                    out=dst[:, ks[0]:ks[-1] + 1, u0:u1],
                    in_=src[:, ks[0]:ks[-1] + 1, u0:u1],
                )

            xb0 = xpool.tile([128, 4 * U], f32r, tag="xb", name="xb0")
            k_first = []
            for p in a_ps:
                k = keep[p] % 4
                if k not in k_first:
                    k_first.append(k)
            x_emits = [(x_chunks[0], (k,)) for k in k_first]
            x_emits += [(ch, (0, 1, 2, 3)) for ch in x_chunks[1:]]
            emits = []
            for i in range(max(len(x_emits), len(w_chunks))):
                if i < len(x_emits):
                    emits.append(("x", x_emits[i]))
                if i < len(w_chunks):
                    emits.append(("w", w_chunks[i]))
            for kind, args in emits:
                if kind == "x":
                    (u0, u1), ks = args
                    dma_x_chunk(xb0, 0, u0, u1, ks)
                else:
                    a0, a1 = args
                    nc.sync.dma_start(
                        out=wsb[:, offs[a0]:offs[a1]],
                        in_=wt_d[:, offs[a0]:offs[a1]],
                    )

            for b in range(b_per):
                if b == 0:
                    xb = xb0
                else:
                    xb = xpool.tile([128, 4 * U], f32r, tag="xb", name=f"xb{b}")
                    nc.sync.dma_start(out=xb[:], in_=xt_d[b])
                t0 = 0
                for nt in nts:
                    pa = pspool.tile([128, 512], f32, tag="pa")
                    if mb_max:
                        pb = pspool.tile([128, 512], f32, tag="pb")
                    for pos, p in enumerate(a_ps):
                        m = Ms[p]
                        j, k = divmod(keep[p], 4)
                        rhs = xb[:, k * U + t0 + j: k * U + t0 + j + nt]
                        ma = min(m, 128)
                        nc.tensor.matmul(
                            pa[:ma, :nt],
                            lhsT=wsb[:, offs[p]: offs[p] + ma],
                            rhs=rhs,
                            start=(pos == 0),
                            stop=(pos == len(a_ps) - 1),
                        )
                        if m > 128:
                            nc.tensor.matmul(
                                pb[:m - 128, :nt],
                                lhsT=wsb[:, offs[p] + 128: offs[p] + m],
                                rhs=rhs,
                                start=(p == b_ps[0]),
                                stop=(p == b_ps[-1]),
                            )
                    ma1 = min(Ms[a_ps[0]], 128)
                    eva = evpool.tile([128, 512], f32, tag="eva")
                    nc.vector.tensor_copy(eva[:ma1, :nt], pa[:ma1, :nt])
                    nc.sync.dma_start(
                        out=out_d[b, 0:ma1, t0:t0 + nt], in_=eva[:ma1, :nt]
                    )
                    if mb_max:
                        evb = evpool.tile([128, 512], f32, tag="evb")
                        nc.vector.tensor_copy(evb[:mb_max, :nt], pb[:mb_max, :nt])
                        nc.sync.dma_start(
                            out=out_d[b, 128:128 + mb_max, t0:t0 + nt],
                            in_=evb[:mb_max, :nt],
                        )
                    t0 += nt
    nc.compile()
    return nc


def _ensure_trace_shims():
    """If run_bass_kernel_spmd is invoked with tracing enabled (e.g. via
    BASS_TRACE=1) it imports antenv.axon_hooks and uploads artifacts to a
    bucket; neither exists in a bare container.  Register a working NTFF
    hook (ctypes into the axon .so) and a no-op uploader so the trace path
    degrades gracefully instead of crashing."""
    import sys

    try:
        import antenv.axon_hooks  # noqa: F401
    except ImportError:
        import contextlib
        import ctypes
        import types

        hook = None
        try:
            lib = ctypes.CDLL("/opt/axon/libaxon_pjrt.so")
            if hasattr(lib, "axon_start_nrt_profile"):
                lib.axon_start_nrt_profile.argtypes = [
                    ctypes.POINTER(ctypes.c_int64),
                    ctypes.c_size_t,
                ]
                lib.axon_start_nrt_profile.restype = ctypes.c_int64
                lib.axon_stop_nrt_profile.argtypes = [ctypes.c_char_p]
                lib.axon_stop_nrt_profile.restype = ctypes.c_int64

                @contextlib.contextmanager
                def _hook(output_dir, device_ids):
                    import jax

                    jax.devices()
                    if device_ids:
                        ids = (ctypes.c_int64 * len(device_ids))(*device_ids)
                        rc = lib.axon_start_nrt_profile(ids, len(device_ids))
                    else:
                        rc = lib.axon_start_nrt_profile(None, 0)
                    if rc != 0:
                        raise RuntimeError(f"axon_start_nrt_profile rc={rc}")
                    try:
                        yield
                    finally:
                        lib.axon_stop_nrt_profile(str(output_dir).encode())

                hook = _hook
        except OSError:
            pass
        mod = types.ModuleType("antenv.axon_hooks")
        mod.get_axon_ntff_profile_hook = lambda: hook
        mod.set_axon_ntff_profile_hook = lambda h: None
        sys.modules["antenv.axon_hooks"] = mod

    try:
        import concourse.bass_utils as _bu

        _orig_upload = _bu.upload_artifacts

        def _safe_upload(tmpdir):
            try:
                return _orig_upload(tmpdir)
            except Exception:
                return "local://unavailable"

        if not getattr(_bu, "_safe_upload_installed", False):
            _bu.upload_artifacts = _safe_upload
            _bu._safe_upload_installed = True
    except Exception:
        pass


def kernel(x, kernels):
    _ensure_trace_shims()
    from concourse.bass_utils import run_bass_kernel_spmd

    hp = _host_prep(x, kernels)
    xt, mode, wexp = hp["xt"], hp["mode"], hp["wexp"]
    C, U, T_out, nbins = hp["C"], hp["U"], hp["T_out"], hp["nbins"]
    B = xt.shape[0]
    assert B % N_CORES == 0
    b_per = B // N_CORES

    if mode == "D":
        nq, nchunks, runs, has_b = hp["meta"]
        runs = [list(r) for r in runs]
        key = ("D", b_per, C, U, T_out, nq, nchunks,
               tuple(tuple(t) for rr in runs for t in rr), has_b)
        if key not in _prog_cache:
            _prog_cache[key] = _build_program_d(
                b_per, C, U, T_out, nq, nchunks, runs, list(has_b)
            )
        nc = _prog_cache[key]
        in_maps = [
            {"xt": xt[c * b_per:(c + 1) * b_per], "wt": hp["wt"], "ew": hp["ew"]}
            for c in range(N_CORES)
        ]
    else:
        keep, Ms, offs = hp["keep"], hp["Ms"], hp["offs"]
        key = ("A", b_per, C, U, T_out, tuple(keep), tuple(Ms))
        if key not in _prog_cache:
            _prog_cache[key] = _build_program_a(b_per, C, U, T_out, keep, Ms, offs)
        nc = _prog_cache[key]
        in_maps = [
            {"xt": xt[c * b_per:(c + 1) * b_per], "wt": hp["wt"]}
            for c in range(N_CORES)
        ]

    res = run_bass_kernel_spmd(nc, in_maps, list(range(N_CORES)))
    parts = [res.results[c]["out"] for c in range(N_CORES)]
    out = np.concatenate(parts, axis=0)  # (B, C, T_out)
    if wexp:
        out = out * np.float32(2.0 ** -wexp)
    return np.ascontiguousarray(
        out.reshape(B, nbins, 2, T_out).transpose(0, 2, 1, 3)
    )


# revision 23
# speedup vs baseline: 2.4609x; 1.0325x over previous
"""CQT (constant-Q transform) kernel for Trainium2, 8 NeuronCores.

Math: out[b, c, t] = sum_l W[c, l] * x_pad[b, t*HOP + l]   (strided conv,
HOP=512, L=11339 taps, C=168 channels = 84 bins x re/im), then reshaped to
(B, 2, n_bins, T_out).

Strategy (two-level factorization, data-parallel over batch):
  - Write l = 512q + s.  Level 1 contracts s: with the polyphase matrix
    Y[s, u] = xp[512u + s], compute G[(c,q), u] = sum_s W[c,512q+s] Y[s,u]
    for the ~815 ACTIVE (channel, hop-block) pairs only (CQT kernels are
    ragged: bin k has ~11339*2^(-k/12) centered taps, so sum_c ceil(l_c/512)
    ~= 815 of 168*23 possible pairs).  Packed into ceil(815/128) = 7 dense
    chunks of 128 pairs -> 28 matmuls per u-sweep at N=512, ~91% PE
    utilization (vs 29% for the naive 128-tap-block decomposition).
  - Shift on evict: PSUM->SBUF copies write Gs[p, t] = G[p, t+q(p)] (pairs
    sharing q form contiguous runs, so each run is one affine copy),
    converting fp32->fp16.
  - Level 2 contracts q: out[c, t] = sum_p E[p, c] Gs[p, t] with one-hot
    E per chunk -- 7+1 matmuls of N=512 per t-tile into the usual
    [C, t] PSUM layout.
  - fp16 x and 2^wexp-scaled fp16 W (undone on host); fp32 PSUM.
  - A formulation (weights stationary, x moving, fp32r) kept as fallback
    if the kernel tensor has no zero raggedness.
"""

import numpy as np

HOP = 512
N_CORES = 8

_prog_cache: dict = {}


def _host_prep(x, kernels):
    x = np.ascontiguousarray(np.asarray(x, dtype=np.float32))
    kernels = np.ascontiguousarray(np.asarray(kernels, dtype=np.float32))
    B, T = x.shape
    nbins, two, Lmax = kernels.shape
    assert two == 2
    C = 2 * nbins
    pad = Lmax // 2
    T_out = (T + 2 * pad - Lmax) // HOP + 1

    nblk_full = -(-Lmax // 128)
    nq = -(-(nblk_full * 128) // 512)
    Wp = np.zeros((C, nq * 512), dtype=np.float32)
    Wp[:, :Lmax] = kernels.reshape(C, Lmax)

    # ---- active (q, c) pairs for the D formulation ----
    seg_nz = (Wp.reshape(C, nq, 512) != 0.0).any(axis=2)  # [C, nq]
    # cluster c>=128 pairs at the end so they occupy the fewest chunks
    # (each chunk holding such pairs costs an extra matmul per t-tile)
    pairs = sorted(
        ((q, c) for q in range(nq) for c in range(C) if seg_nz[c, q]),
        key=lambda qc: (qc[1] >= 128, qc[0], qc[1]),
    )
    npairs = len(pairs)
    nchunks = -(-npairs // 128)
    npad = nchunks * 128

    # ---- cost model: D (two-level) vs A (tap-block, W-stationary) ----
    nzb = (Wp[:, :nblk_full * 128].reshape(C, nblk_full, 128) != 0.0).any(axis=2)
    Msb, keepb = [], []
    for i in range(nblk_full):
        idx = np.where(nzb[:, i])[0]
        if len(idx):
            keepb.append(i)
            Msb.append(int(idx[-1]) + 1)
    j_max_a = (max(keepb) // 4) if keepb else 0
    U = T_out + max(nq - 1, j_max_a)
    cost_d = nchunks * 4 * U + (nchunks + (1 if C > 128 else 0)) * T_out
    cost_a = (len(keepb) + sum(1 for m in Msb if m > 128)) * T_out
    mode = "D" if (cost_d < cost_a and C <= 256) else "A"

    if mode == "D":
        U = T_out + nq - 1
        wmax = float(np.abs(Wp).max())
        wexp = int(np.floor(np.log2(0.25 / wmax))) if wmax > 0 else 0
        scale = np.float32(2.0 ** wexp)
        # wq[r, k, p] = Wp[c, 512q + 128k + r] * scale
        wq = np.zeros((128, 4, npad), dtype=np.float16)
        ee = np.zeros((128, nchunks * C), dtype=np.float16)
        runs = [[] for _ in range(nchunks)]
        has_b = [False] * nchunks
        for p, (q, c) in enumerate(pairs + [(0, 0)] * (npad - npairs)):
            m, r = divmod(p, 128)
            if p < npairs:
                wq[:, :, p] = (Wp[c, 512 * q: 512 * (q + 1)] * scale).reshape(4, 128).T
                ee[r, m * C + c] = 2.0 ** -wexp  # folds the W-scale undo into L2
                if c >= 128:
                    has_b[m] = True
            if runs[m] and runs[m][-1][2] == q:
                runs[m][-1] = (runs[m][-1][0], r + 1, q)
            else:
                runs[m].append((r, r + 1, q))
        wq = np.ascontiguousarray(wq.reshape(128, 4 * npad))
        wexp = 0  # already undone via ee
        xdt = np.float16
        meta = (nq, nchunks, tuple(
            tuple(rr) for rr in ((tuple(t) for t in runs[m]) for m in range(nchunks))
        ), tuple(has_b))
        wt, keep, Ms, offs = wq, None, None, None
        ew = ee
    else:
        wexp = 0
        # ragged 128-tap blocks, desc active-prefix order (A path)
        keep = np.asarray(keepb, dtype=np.int64)
        Ms = np.asarray(Msb, dtype=np.int64)
        order = np.argsort(-Ms, kind="stable")
        keep = keep[order]
        Ms = Ms[order]
        wblk = Wp[:, :nblk_full * 128].reshape(C, nblk_full, 128)
        wt = np.ascontiguousarray(
            np.concatenate([wblk[:m, i, :].T for i, m in zip(keep, Ms)], axis=1)
        )
        offs = np.concatenate([[0], np.cumsum(Ms)]).tolist()
        keep = keep.tolist()
        Ms = Ms.tolist()
        U = T_out + max(keep) // 4
        xdt = np.float32
        meta = None
        ew = None

    xpad_len = 512 * U
    assert xpad_len >= pad + T, (xpad_len, pad + T)
    xp = np.zeros((B, xpad_len), dtype=xdt)
    xp[:, pad:pad + T] = x.astype(xdt)
    # xt[b, r, k*U + u] = xp[b, 512u + 128k + r]
    xt = np.ascontiguousarray(
        xp.reshape(B, U, 4, 128).transpose(0, 3, 2, 1).reshape(B, 128, 4 * U)
    )
    return dict(
        xt=xt, wt=wt, ew=ew, keep=keep, Ms=Ms, offs=offs, C=C, U=U,
        T_out=T_out, nbins=nbins, mode=mode, wexp=wexp, meta=meta,
    )


def _tiles(total, step):
    return [(t0, min(step, total - t0)) for t0 in range(0, total, step)]


def _build_program_d(b_per, C, U, T_out, nq, nchunks, runs, has_b):
    import concourse.mybir as mybir
    import concourse.tile as tile
    from concourse import bacc

    f32 = mybir.dt.float32
    f16 = mybir.dt.float16
    npad = nchunks * 128
    cb = C - 128 if C > 128 else 0
    u_tiles = _tiles(U, 512)
    t_tiles = _tiles(T_out, 512)
    b_chunks = [m for m in range(nchunks) if has_b[m]]

    nc = bacc.Bacc(
        "TRN2",
        target_bir_lowering=False,
        debug=False,
        enable_asserts=True,
        num_devices=N_CORES,
    )
    xt_d = nc.dram_tensor("xt", [b_per, 128, 4 * U], f16, kind="ExternalInput").ap()
    wt_d = nc.dram_tensor("wt", [128, 4 * npad], f16, kind="ExternalInput").ap()
    ew_d = nc.dram_tensor("ew", [128, nchunks * C], f16, kind="ExternalInput").ap()
    # fp16 output (cast to fp32 on host): halves the writeback DMA
    out_d = nc.dram_tensor("out", [b_per, C, T_out], f16, kind="ExternalOutput").ap()

    with tile.TileContext(nc) as tc:
        with (
            tc.tile_pool(name="wpool", bufs=1) as wpool,
            tc.tile_pool(name="xpool", bufs=2) as xpool,
            tc.tile_pool(name="gspool", bufs=2) as gspool,
            tc.tile_pool(name="gtpool", bufs=4) as gtpool,
            tc.tile_pool(name="evpool", bufs=3) as evpool,
            tc.tile_pool(name="ps1pool", bufs=4, space="PSUM") as ps1pool,
            tc.tile_pool(name="ps2pool", bufs=2, space="PSUM") as ps2pool,
        ):
            wsb = wpool.tile([128, 4 * npad], f16)
            esb = wpool.tile([128, nchunks * C], f16)
            wk = wsb.rearrange("r (k p) -> r k p", k=4)

            def dma_x_chunk(xb_tile, b, u0, u1, eng):
                src = xt_d[b].rearrange("r (k u) -> r k u", k=4)
                dst = xb_tile.rearrange("r (k u) -> r k u", k=4)
                eng.dma_start(out=dst[:, :, u0:u1], in_=src[:, :, u0:u1])

            # weights + selection matrices (scalar/gpsimd queues are still
            # empty at program start, so spread the initial loads)
            for k in range(4):
                nc.scalar.dma_start(
                    out=wk[:, k, :], in_=wt_d.rearrange("r (k p) -> r k p", k=4)[:, k, :]
                )
            nc.gpsimd.dma_start(out=esb[:], in_=ew_d[:])

            # dedicated queues: sync carries I/O (x in, out back); scalar +
            # gpsimd carry the Gs shift copies.  Queues are FIFO, so mixing
            # dependency-stalled shift DMAs with I/O causes head-of-line
            # blocking.
            dma_engs = [nc.scalar, nc.gpsimd]
            rr_state = [0]

            def level1_chunk(xb, gs, m):
                # compute chunk m's G over the full U, staged in fp16, then
                # apply the per-run q-shift with one big SBUF->SBUF DMA per
                # run (engines need 32-aligned partition bases; DMA doesn't),
                # round-robined across queues so they run in parallel
                gt = gtpool.tile([128, U], f16, tag="gt")
                for (u0, nu) in u_tiles:
                    ps = ps1pool.tile([128, 512], f32, tag="ps1")
                    for k in range(4):
                        nc.tensor.matmul(
                            ps[:, :nu],
                            lhsT=wk[:, k, m * 128:(m + 1) * 128],
                            rhs=xb[:, k * U + u0: k * U + u0 + nu],
                            start=(k == 0),
                            stop=(k == 3),
                        )
                    nc.vector.tensor_copy(gt[:, u0:u0 + nu], ps[:, :nu])
                for (r0, r1, q) in runs[m]:
                    eng = dma_engs[rr_state[0] % len(dma_engs)]
                    rr_state[0] += 1
                    eng.dma_start(
                        out=gs[r0:r1, m * T_out: (m + 1) * T_out],
                        in_=gt[r0:r1, q: q + T_out],
                    )

            def level2(gs, b, t0, nt):
                pa = ps2pool.tile([128, 512], f32, tag="pa")
                if cb:
                    pb = ps2pool.tile([128, 512], f32, tag="pb")
                for m in range(nchunks):
                    rhs = gs[:, m * T_out + t0: m * T_out + t0 + nt]
                    nc.tensor.matmul(
                        pa[:min(C, 128), :nt],
                        lhsT=esb[:, m * C: m * C + min(C, 128)],
                        rhs=rhs,
                        start=(m == 0),
                        stop=(m == nchunks - 1),
                    )
                    if cb and has_b[m]:
                        nc.tensor.matmul(
                            pb[:cb, :nt],
                            lhsT=esb[:, m * C + 128: m * C + C],
                            rhs=rhs,
                            start=(m == b_chunks[0]),
                            stop=(m == b_chunks[-1]),
                        )
                # evictions on the scalar engine so they don't queue behind
                # the L1 casts on the DVE
                eva = evpool.tile([128, 512], f16, tag="eva")
                nc.scalar.copy(eva[:min(C, 128), :nt], pa[:min(C, 128), :nt])
                nc.sync.dma_start(
                    out=out_d[b, 0:min(C, 128), t0:t0 + nt],
                    in_=eva[:min(C, 128), :nt],
                )
                if cb:
                    evb = evpool.tile([128, 512], f16, tag="evb")
                    nc.scalar.copy(evb[:cb, :nt], pb[:cb, :nt])
                    nc.sync.dma_start(
                        out=out_d[b, 128:C, t0:t0 + nt], in_=evb[:cb, :nt]
                    )

            # software pipeline: emit L1(b+1) before L2(b) so the in-order
            # PE queue has work while batch b's shift DMAs drain
            gs_by_b = {}
            for b in range(b_per):
                xb = xpool.tile([128, 4 * U], f16, tag="xb", name=f"xb{b}")
                for (u0, nu) in u_tiles:
                    dma_x_chunk(xb, b, u0, u0 + nu, nc.sync)
                gs = gspool.tile([128, nchunks * T_out], f16, tag="gs")
                gs_by_b[b] = gs
                for m in range(nchunks):
                    level1_chunk(xb, gs, m)
                if b > 0:
                    for (t0, nt) in t_tiles:
                        level2(gs_by_b[b - 1], b - 1, t0, nt)
            for (t0, nt) in t_tiles:
                level2(gs_by_b[b_per - 1], b_per - 1, t0, nt)
    nc.compile()
    return nc


def _build_program_a(b_per, C, U, T_out, keep, Ms, offs):
    """A formulation (fallback for dense kernels): weights stationary,
    x moving, fp32r; out[b, c, t]."""
    import concourse.mybir as mybir
    import concourse.tile as tile
    from concourse import bacc

    f32 = mybir.dt.float32
    f32r = mybir.dt.float32r
    nblk = len(keep)
    sum_m = offs[-1]
    mb_max = max(max(Ms) - 128, 0)
    nts = [512] * (T_out // 512) + ([T_out % 512] if T_out % 512 else [])
    a_ps = list(range(nblk))
    b_ps = [p for p in a_ps if Ms[p] > 128]
    j_max = max(keep) // 4
    w_budgets = [192, 256, 512] + [704] * nblk
    w_chunks = []
    p0 = 0
    while p0 < nblk:
        budget = w_budgets[len(w_chunks)]
        p1 = p0 + 1
        while p1 < nblk and offs[p1 + 1] - offs[p0] <= budget:
            p1 += 1
        w_chunks.append((p0, p1))
        p0 = p1
    x_stops = []
    t0 = 0
    for nt in nts:
        x_stops.append(min(t0 + nt + j_max + 1, U))
        t0 += nt
    x_stops[-1] = U
    x_chunks = []
    u0 = 0
    for u1 in x_stops:
        if u1 > u0:
            x_chunks.append((u0, u1))
            u0 = u1

    nc = bacc.Bacc(
        "TRN2",
        target_bir_lowering=False,
        debug=False,
        enable_asserts=True,
        num_devices=N_CORES,
    )
    xt_d = nc.dram_tensor("xt", [b_per, 128, 4 * U], f32r, kind="ExternalInput").ap()
    wt_d = nc.dram_tensor("wt", [128, sum_m], f32r, kind="ExternalInput").ap()
    out_d = nc.dram_tensor("out", [b_per, C, T_out], f32, kind="ExternalOutput").ap()

    with tile.TileContext(nc) as tc:
        with (
            tc.tile_pool(name="wpool", bufs=1) as wpool,
            tc.tile_pool(name="xpool", bufs=2) as xpool,
            tc.tile_pool(name="evpool", bufs=3) as evpool,
            tc.tile_pool(name="pspool", bufs=2, space="PSUM") as pspool,
        ):
            wsb = wpool.tile([128, sum_m], f32r)

            def dma_x_chunk(xb_tile, b, u0, u1, ks):
                src = xt_d[b].rearrange("r (k u) -> r k u", k=4)
                dst = xb_tile.rearrange("r (k u) -> r k u", k=4)
                nc.sync.dma_start(
                    out=dst[:, ks[0]:ks[-1] + 1, u0:u1],
                    in_=src[:, ks[0]:ks[-1] + 1, u0:u1],
                )

            xb0 = xpool.tile([128, 4 * U], f32r, tag="xb", name="xb0")
            k_first = []
            for p in a_ps:
                k = keep[p] % 4
                if k not in k_first:
                    k_first.append(k)
            x_emits = [(x_chunks[0], (k,)) for k in k_first]
            x_emits += [(ch, (0, 1, 2, 3)) for ch in x_chunks[1:]]
            emits = []
            for i in range(max(len(x_emits), len(w_chunks))):
                if i < len(x_emits):
                    emits.append(("x", x_emits[i]))
                if i < len(w_chunks):
                    emits.append(("w", w_chunks[i]))
            for kind, args in emits:
                if kind == "x":
                    (u0, u1), ks = args
                    dma_x_chunk(xb0, 0, u0, u1, ks)
                else:
                    a0, a1 = args
                    nc.sync.dma_start(
                        out=wsb[:, offs[a0]:offs[a1]],
                        in_=wt_d[:, offs[a0]:offs[a1]],
                    )

            for b in range(b_per):
                if b == 0:
                    xb = xb0
                else:
                    xb = xpool.tile([128, 4 * U], f32r, tag="xb", name=f"xb{b}")
                    nc.sync.dma_start(out=xb[:], in_=xt_d[b])
                t0 = 0
                for nt in nts:
                    pa = pspool.tile([128, 512], f32, tag="pa")
                    if mb_max:
                        pb = pspool.tile([128, 512], f32, tag="pb")
                    for pos, p in enumerate(a_ps):
                        m = Ms[p]
                        j, k = divmod(keep[p], 4)
                        rhs = xb[:, k * U + t0 + j: k * U + t0 + j + nt]
                        ma = min(m, 128)
                        nc.tensor.matmul(
                            pa[:ma, :nt],
                            lhsT=wsb[:, offs[p]: offs[p] + ma],
                            rhs=rhs,
                            start=(pos == 0),
                            stop=(pos == len(a_ps) - 1),
                        )
                        if m > 128:
                            nc.tensor.matmul(
                                pb[:m - 128, :nt],
                                lhsT=wsb[:, offs[p] + 128: offs[p] + m],
                                rhs=rhs,
                                start=(p == b_ps[0]),
                                stop=(p == b_ps[-1]),
                            )
                    ma1 = min(Ms[a_ps[0]], 128)
                    eva = evpool.tile([128, 512], f32, tag="eva")
                    nc.vector.tensor_copy(eva[:ma1, :nt], pa[:ma1, :nt])
                    nc.sync.dma_start(
                        out=out_d[b, 0:ma1, t0:t0 + nt], in_=eva[:ma1, :nt]
                    )
                    if mb_max:
                        evb = evpool.tile([128, 512], f32, tag="evb")
                        nc.vector.tensor_copy(evb[:mb_max, :nt], pb[:mb_max, :nt])
                        nc.sync.dma_start(
                            out=out_d[b, 128:128 + mb_max, t0:t0 + nt],
                            in_=evb[:mb_max, :nt],
                        )
                    t0 += nt
    nc.compile()
    return nc


def _ensure_trace_shims():
    """If run_bass_kernel_spmd is invoked with tracing enabled (e.g. via
    BASS_TRACE=1) it imports antenv.axon_hooks and uploads artifacts to a
    bucket; neither exists in a bare container.  Register a working NTFF
    hook (ctypes into the axon .so) and a no-op uploader so the trace path
    degrades gracefully instead of crashing."""
    import sys

    try:
        import antenv.axon_hooks  # noqa: F401
    except ImportError:
        import contextlib
        import ctypes
        import types

        hook = None
        try:
            lib = ctypes.CDLL("/opt/axon/libaxon_pjrt.so")
            if hasattr(lib, "axon_start_nrt_profile"):
                lib.axon_start_nrt_profile.argtypes = [
                    ctypes.POINTER(ctypes.c_int64),
                    ctypes.c_size_t,
                ]
                lib.axon_start_nrt_profile.restype = ctypes.c_int64
                lib.axon_stop_nrt_profile.argtypes = [ctypes.c_char_p]
                lib.axon_stop_nrt_profile.restype = ctypes.c_int64

                @contextlib.contextmanager
                def _hook(output_dir, device_ids):
                    import jax

                    jax.devices()
                    if device_ids:
                        ids = (ctypes.c_int64 * len(device_ids))(*device_ids)
                        rc = lib.axon_start_nrt_profile(ids, len(device_ids))
                    else:
                        rc = lib.axon_start_nrt_profile(None, 0)
                    if rc != 0:
                        raise RuntimeError(f"axon_start_nrt_profile rc={rc}")
                    try:
                        yield
                    finally:
                        lib.axon_stop_nrt_profile(str(output_dir).encode())

                hook = _hook
        except OSError:
            pass
        mod = types.ModuleType("antenv.axon_hooks")
        mod.get_axon_ntff_profile_hook = lambda: hook
        mod.set_axon_ntff_profile_hook = lambda h: None
        sys.modules["antenv.axon_hooks"] = mod

    try:
        import concourse.bass_utils as _bu

        _orig_upload = _bu.upload_artifacts

        def _safe_upload(tmpdir):
            try:
                return _orig_upload(tmpdir)
            except Exception:
                return "local://unavailable"

        if not getattr(_bu, "_safe_upload_installed", False):
            _bu.upload_artifacts = _safe_upload
            _bu._safe_upload_installed = True
    except Exception:
        pass


def kernel(x, kernels):
    _ensure_trace_shims()
    from concourse.bass_utils import run_bass_kernel_spmd

    hp = _host_prep(x, kernels)
    xt, mode, wexp = hp["xt"], hp["mode"], hp["wexp"]
    C, U, T_out, nbins = hp["C"], hp["U"], hp["T_out"], hp["nbins"]
    B = xt.shape[0]
    assert B % N_CORES == 0
    b_per = B // N_CORES

    if mode == "D":
        nq, nchunks, runs, has_b = hp["meta"]
        runs = [list(r) for r in runs]
        key = ("D", b_per, C, U, T_out, nq, nchunks,
               tuple(tuple(t) for rr in runs for t in rr), has_b)
        if key not in _prog_cache:
            _prog_cache[key] = _build_program_d(
                b_per, C, U, T_out, nq, nchunks, runs, list(has_b)
            )
        nc = _prog_cache[key]
        in_maps = [
            {"xt": xt[c * b_per:(c + 1) * b_per], "wt": hp["wt"], "ew": hp["ew"]}
            for c in range(N_CORES)
        ]
    else:
        keep, Ms, offs = hp["keep"], hp["Ms"], hp["offs"]
        key = ("A", b_per, C, U, T_out, tuple(keep), tuple(Ms))
        if key not in _prog_cache:
            _prog_cache[key] = _build_program_a(b_per, C, U, T_out, keep, Ms, offs)
        nc = _prog_cache[key]
        in_maps = [
            {"xt": xt[c * b_per:(c + 1) * b_per], "wt": hp["wt"]}
            for c in range(N_CORES)
        ]

    res = run_bass_kernel_spmd(nc, in_maps, list(range(N_CORES)))
    parts = [res.results[c]["out"] for c in range(N_CORES)]
    out = np.concatenate(parts, axis=0).astype(np.float32)  # (B, C, T_out)
    if wexp:
        out = out * np.float32(2.0 ** -wexp)
    return np.ascontiguousarray(
        out.reshape(B, nbins, 2, T_out).transpose(0, 2, 1, 3)
    )
